# revision 17
# baseline (speedup 1.0000x reference)
"""DDSP generator Bass kernel for Trainium2, 8-core data parallel.

Sharding: batch 16 -> 8 cores x 2 examples each. Weights replicated.
Per core:
  stage1: main conv stack (fp32 PE) -> h; osc head -> l (amp^2), f (Hz/SR)
  osc bank, per 384-sample resize segment (plus two 192 edge segments):
      lerp (DVE/Pool tensor_scalar, per-partition scale/bias) ->
      custom DVE op (clip + cumsum + wrap to [-0.5, 0.5] cycles, one pass) ->
      ACT Sin (per group of 4 units) -> fp16 -> m=2 PE reduce matmul with
      lhsT = [l_lo | dl], 4 segments packed per PSUM bank via tile_position ->
      direct PSUM -> DRAM DMA (partition-strided, one per group).
  noise branch: 4x (2x-upsample conv k7) via even/odd stride trick
      (host-combined 4-tap weights), fp8e4m3 DoubleRow matmuls (0.5 cyc/row,
      contraction 256); activations quantized to fp8 between layers; last
      layer kept fp16 for the head conv (duplicated 34-col weights) +
      Square -> n_l on partitions 0..33.
  noise FFT: rfft/irfft as fp32r DFT matmuls, filter on DVE, overlap-add.
Host: recombine the two reduce rows with the lerp-weight pattern, pad,
      add noise, normalize, crop (O(output) numpy work only).
"""

import numpy as np
from contextlib import ExitStack

import ml_dtypes

import concourse.bass as bass
import concourse.tile as tile
from concourse import bacc, mybir
from concourse import bass_utils
from concourse import dve_ops
from concourse.dve_spec import Spec, Src0, Src1, C0, C1, C2, scan, minn, maxx, AluOp, lower
from concourse.dve_uop import DveOpSpec

F32 = mybir.dt.float32
F16 = mybir.dt.float16
F8 = mybir.dt.float8e4
U8 = mybir.dt.uint8
F32R = mybir.dt.float32r
AF = mybir.ActivationFunctionType
ALU = mybir.AluOpType
DR = mybir.MatmulPerfMode.DoubleRow

SR = 11025.0
UP_LEN = 24576
TOTAL = 16384
WIN = 32
FRAMES = 1024
CROP = 4096
B = 16
NCORES = 8
BPC = 2
T0 = 64
SEG = 384
NSEG = 63
EDGE = 192
NUNITS = NSEG + 2
LO_U = 20.0 / SR
HI_U = 0.5
MAGIC = 12582912.0

_CENTERS = np.geomspace(20.0, SR / 2.0 - 20.0, 128).astype(np.float32)
_ERBS = (_CENTERS * np.float32(0.108) + np.float32(24.7)).astype(np.float32)


def _osc_ref(in0, in1, s0, s1, imm2):
    v = np.minimum(np.maximum(in0, np.float32(s0)), np.float32(s1)).astype(np.float32)
    u = np.cumsum(v.astype(np.float64), axis=-1).astype(np.float32)
    y = (u + in1).astype(np.float32)
    r = ((y + np.float32(imm2)) - np.float32(imm2)).astype(np.float32)
    return (y - r).astype(np.float32)


def _register_osc_op():
    if hasattr(dve_ops, "CUSTOM_DVE_OPS_BY_NAME") and \
            "OSC_PHASE_ANT" in dve_ops.CUSTOM_DVE_OPS_BY_NAME:
        return dve_ops.CUSTOM_DVE_OPS_BY_NAME["OSC_PHASE_ANT"]
    body_v = minn(maxx(Src0, C0), C1)
    body_u = scan(AluOp.ADD, body_v)
    body_y = body_u + Src1
    body = body_y - ((body_y + C2) - C2)
    spec = Spec(body=body, reference=_osc_ref)
    sha = {}
    for ver in ("v3",):
        s = DveOpSpec(name="OSC_PHASE_ANT", opcode=1, uops=lower(spec, ver=ver),
                      rd1_en=True)
        sha[ver] = s.sha(ver)
    op = dve_ops.DveOp("OSC_PHASE_ANT", spec, subdim=False, uops_sha=sha)
    dve_ops.OPS.append(op)
    dve_ops.CUSTOM_DVE_SPECS[op.name] = op.spec
    dve_ops._SUB_OPCODE_FOR_NAME[op.name] = max(dve_ops._SUB_OPCODE_FOR_NAME.values()) + 1
    if not hasattr(dve_ops, "CUSTOM_DVE_OPS_BY_NAME"):
        dve_ops.CUSTOM_DVE_OPS_BY_NAME = {}
    dve_ops.CUSTOM_DVE_OPS_BY_NAME[op.name] = op
    return op


def _osc_groups():
    """Per-example unit grouping: 16 groups of SEG units (last has 3) plus
    one edge group [unit0, unit64]. Returns list of (glist, is_edge) where
    glist entries are (j, c0, wdt)."""
    groups = []
    for g in range(16):
        js = [1 + 4 * g + i for i in range(4) if 1 + 4 * g + i <= 63]
        groups.append(([(j, EDGE + SEG * (j - 1), SEG) for j in js], False))
    groups.append(([(0, 0, EDGE), (NUNITS - 1, UP_LEN - EDGE, EDGE)], True))
    return groups


_BUILD_CACHE = {}


def _build_program():
    if "nc" in _BUILD_CACHE:
        return _BUILD_CACHE["nc"]
    osc_op = _register_osc_op()

    nc = bacc.Bacc("TRN2", target_bir_lowering=False, debug=False, num_devices=1)

    dI = lambda n, s, dt=F32: nc.dram_tensor(n, s, dt, kind="ExternalInput").ap()
    dO = lambda n, s, dt=F32: nc.dram_tensor(n, s, dt, kind="ExternalOutput").ap()

    x3h = dI("x3h", [BPC, 256, T0], F16)
    x3l = dI("x3l", [BPC, 256, T0], F16)
    noi = dI("noi", [BPC, FRAMES, WIN])
    wm0 = [dI(f"wm0{s}", [256, 512], F16) for s in "hl"]
    wmL = [[dI(f"wm{i}{s}", [512, 3, 512], F16) for s in "hl"] for i in (1, 2, 3)]
    wfq = [dI(f"wfq{s}", [512, 256], F16) for s in "hl"]
    wn8 = [dI(f"wn8_{l}", [2, 128, 4 * 2 * 2 * 512], U8) for l in range(4)]
    wnh = dI("wnh", [512, 34], F16)                       # head, duplicated cols
    bnl = dI("bnl", [128, 16])
    wt = dI("wt", [128, SEG])
    fcat = dI("fcat", [WIN, 34])
    gmat = dI("gmat", [34, WIN])
    cesc = dI("cesc", [128, 1])
    cebi = dI("cebi", [128, 1])

    h_out = dO("h_out", [BPC, 128, 17 * SEG], F16)
    n_out = dO("n_out", [16 * BPC, FRAMES])

    with tile.TileContext(nc) as tc, ExitStack() as ctx:
        cpool = ctx.enter_context(tc.tile_pool(name="consts", bufs=1))
        apool = ctx.enter_context(tc.tile_pool(name="acts", bufs=1))
        fpool = ctx.enter_context(tc.tile_pool(name="fft", bufs=1))
        opool = ctx.enter_context(tc.tile_pool(name="osc", bufs=1))
        w1pool = ctx.enter_context(tc.tile_pool(name="w1", bufs=2))
        w2pool = ctx.enter_context(tc.tile_pool(name="w2", bufs=2))
        ps_mm = ctx.enter_context(tc.tile_pool(name="psmm", bufs=4, space="PSUM"))
        ps_osc = ctx.enter_context(tc.tile_pool(name="psosc", bufs=2, space="PSUM"))
        ps_fft = ctx.enter_context(tc.tile_pool(name="psfft", bufs=2, space="PSUM"))

        wt_t = cpool.tile([128, SEG], F32)
        nc.sync.dma_start(wt_t[:], wt[:])
        cesc_t = cpool.tile([128, 1], F32)
        nc.sync.dma_start(cesc_t[:], cesc[:])
        cebi_t = cpool.tile([128, 1], F32)
        nc.sync.dma_start(cebi_t[:], cebi[:])
        fcat_t = cpool.tile([WIN, 34], F32R)
        nc.sync.dma_start(fcat_t[:], fcat[:].bitcast(F32R))
        gmat_t = cpool.tile([34, WIN], F32R)
        nc.sync.dma_start(gmat_t[:], gmat[:].bitcast(F32R))
        bnl_t = cpool.tile([128, 16], F32)
        nc.sync.dma_start(bnl_t[:], bnl[:])

        # ================= stage 1: main conv stack =================
        x_t = []
        for k in range(2):
            xth = apool.tile([128, BPC, T0], F16, tag=f"xh{k}")
            nc.sync.dma_start(xth[:], x3h[:, 128 * k:128 * (k + 1), :].rearrange("b c t -> c b t"))
            xtl = apool.tile([128, BPC, T0], F16, tag=f"xl{k}")
            nc.sync.dma_start(xtl[:], x3l[:, 128 * k:128 * (k + 1), :].rearrange("b c t -> c b t"))
            x_t.append((xth, xtl))

        wm0_t = []
        for k in range(2):
            wh = w1pool.tile([128, 512], F16, tag=f"wm0h_{k}")
            nc.sync.dma_start(wh[:], wm0[0][128 * k:128 * (k + 1), :])
            wl_ = w1pool.tile([128, 512], F16, tag=f"wm0l_{k}")
            nc.sync.dma_start(wl_[:], wm0[1][128 * k:128 * (k + 1), :])
            wm0_t.append((wh, wl_))

        NCOL = BPC * T0
        h1 = []
        for m in range(4):
            pm = ps_mm.tile([128, 512], F32, tag="pconv")
            i_mm = 0
            for k in range(2):
                ms = slice(128 * m, 128 * (m + 1))
                for lh, rh in ((0, 0), (0, 1), (1, 0)):
                    nc.tensor.matmul(pm[:, 0:NCOL], wm0_t[k][lh][:, ms],
                                     x_t[k][rh][:],
                                     start=(i_mm == 0), stop=(i_mm == 5))
                    i_mm += 1
            ht = apool.tile([128, BPC, 66], F32, tag=f"hA{m}")
            nc.gpsimd.memset(ht[:, :, 0:1], 0.0)
            nc.gpsimd.memset(ht[:, :, 65:66], 0.0)
            nc.scalar.activation(ht[:, :, 1:65],
                                 pm[:, 0:NCOL].rearrange("c (b t) -> c b t", b=BPC),
                                 AF.Prelu, bias=0.0, scale=1.0, alpha=0.2)
            h1.append(ht)

        def split16(tiles, PAD, WID, tagp):
            # fp32 h tiles -> (hi, lo) fp16 pairs, pads included (zero)
            out = []
            for k, ht in enumerate(tiles):
                hh = apool.tile([128, BPC, WID], F16, tag=f"{tagp}h{k}")
                nc.vector.tensor_copy(hh[:], ht[:])
                hl = apool.tile([128, BPC, WID], F16, tag=f"{tagp}l{k}")
                nc.vector.tensor_tensor(hl[:], ht[:], hh[:], ALU.subtract)
                out.append((hh, hl))
            return out

        hcur = split16(h1, 1, 66, "sA")
        for li in range(3):
            wl = []
            for k in range(4):
                wh = w1pool.tile([128, 3 * 512], F16, tag=f"wmLh_{k}")
                nc.sync.dma_start(wh[:], wmL[li][0][128 * k:128 * (k + 1), :, :]
                                  .rearrange("c a o -> c (a o)"))
                wlo = w1pool.tile([128, 3 * 512], F16, tag=f"wmLl_{k}")
                nc.sync.dma_start(wlo[:], wmL[li][1][128 * k:128 * (k + 1), :, :]
                                  .rearrange("c a o -> c (a o)"))
                wl.append((wh, wlo))
            last = li == 2
            PAD = 2 if last else 1
            WID = T0 + 2 * PAD
            tagp = "hB" if li % 2 == 0 else "hA"
            hnxt = []
            for m in range(4):
                pm = ps_mm.tile([128, 512], F32, tag="pconv")
                i_mm = 0
                for k in range(4):
                    for tap in range(3):
                        wsl = slice(512 * tap + 128 * m, 512 * tap + 128 * (m + 1))
                        for lh, rh in ((0, 0), (0, 1), (1, 0)):
                            nc.tensor.matmul(
                                pm[:, 0:NCOL],
                                wl[k][lh][:, wsl],
                                hcur[k][rh][:, :, tap:tap + T0],
                                start=(i_mm == 0), stop=(i_mm == 35))
                            i_mm += 1
                ht = apool.tile([128, BPC, WID], F32,
                                tag=(f"h4_{m}" if last else f"{tagp}{m}"))
                nc.gpsimd.memset(ht[:, :, 0:PAD], 0.0)
                nc.gpsimd.memset(ht[:, :, PAD + T0:WID], 0.0)
                nc.scalar.activation(ht[:, :, PAD:PAD + T0],
                                     pm[:, 0:NCOL].rearrange("c (b t) -> c b t", b=BPC),
                                     AF.Prelu, bias=0.0, scale=1.0, alpha=0.2)
                hnxt.append(ht)
            hcur = split16(hnxt, PAD, WID, "sB" if li % 2 == 0 else "sA")
            if last:
                h4 = hnxt   # 4 x [128, BPC, 68] fp32, pad 2
        h4s = hcur

        wfq_t = []
        for k in range(4):
            wh = w1pool.tile([128, 256], F16, tag=f"wfqh{k}")
            nc.sync.dma_start(wh[:], wfq[0][128 * k:128 * (k + 1), :])
            wlo = w1pool.tile([128, 256], F16, tag=f"wfql{k}")
            nc.sync.dma_start(wlo[:], wfq[1][128 * k:128 * (k + 1), :])
            wfq_t.append((wh, wlo))
        l_sb = apool.tile([128, BPC, T0], F32, tag="l_sb")
        f_sb = apool.tile([128, BPC, T0], F32, tag="f_sb")
        for m in range(2):
            pm = ps_mm.tile([128, 512], F32, tag="pconv")
            i_mm = 0
            for k in range(4):
                ms = slice(128 * m, 128 * (m + 1))
                for lh, rh in ((0, 0), (0, 1), (1, 0)):
                    nc.tensor.matmul(pm[:, 0:NCOL], wfq_t[k][lh][:, ms],
                                     h4s[k][rh][:, :, 2:2 + T0],
                                     start=(i_mm == 0), stop=(i_mm == 11))
                    i_mm += 1
            if m == 0:
                nc.scalar.activation(l_sb[:],
                                     pm[:, 0:NCOL].rearrange("c (b t) -> c b t", b=BPC),
                                     AF.Square)
            else:
                tanh_t = apool.tile([128, BPC, T0], F32, tag="tanh")
                nc.scalar.activation(tanh_t[:],
                                     pm[:, 0:NCOL].rearrange("c (b t) -> c b t", b=BPC),
                                     AF.Tanh)
                nc.scalar.activation(f_sb[:], tanh_t[:],
                                     AF.Identity, bias=cebi_t[:], scale=cesc_t[:])

        # ================= osc prep =================
        flo_u, df_u, c_u, l2_u = [], [], [], []
        for ex in range(BPC):
            f_ex = f_sb[:, ex, :]
            l_ex = l_sb[:, ex, :]

            flo = apool.tile([128, NUNITS], F32, tag=f"flo{ex}")
            nc.vector.tensor_copy(flo[:, 0:1], f_ex[:, 0:1])
            nc.vector.tensor_copy(flo[:, 1:65], f_ex[:, 0:64])
            dfu = apool.tile([128, NUNITS], F32, tag=f"dfu{ex}")
            nc.gpsimd.memset(dfu[:, 0:1], 0.0)
            nc.gpsimd.memset(dfu[:, 64:65], 0.0)
            nc.gpsimd.tensor_tensor(dfu[:, 1:64], f_ex[:, 1:64], f_ex[:, 0:63], ALU.subtract)

            l2t = apool.tile([128, NUNITS, 2], F16, tag=f"l2{ex}")
            nc.vector.tensor_copy(l2t[:, 0:1, 0], l_ex[:, 0:1])
            nc.vector.tensor_copy(l2t[:, 1:65, 0], l_ex[:, 0:64])
            nc.gpsimd.memset(l2t[:, 0:1, 1], 0.0)
            nc.gpsimd.memset(l2t[:, 64:65, 1], 0.0)
            nc.gpsimd.tensor_tensor(l2t[:, 1:64, 1], l_ex[:, 1:64], l_ex[:, 0:63], ALU.subtract)

            a = f_ex[:, 0:63]
            b_ = f_ex[:, 1:64]

            def T63(tag):
                return apool.tile([128, 63], F32, tag=tag, name=tag)

            alo = T63("p_alo")
            nc.vector.tensor_tensor(alo[:], a, b_, ALU.min)
            ahi = T63("p_ahi")
            nc.vector.tensor_tensor(ahi[:], a, b_, ALU.max)
            dd = T63("p_dd")
            nc.vector.tensor_tensor(dd[:], ahi[:], alo[:], ALU.subtract)
            ddc = T63("p_ddc")
            nc.vector.tensor_scalar(ddc[:], dd[:], 1e-30, None, ALU.max)
            inv = T63("p_inv")
            nc.vector.reciprocal(inv[:], ddc[:])
            dd768 = T63("p_dd768")
            nc.vector.tensor_scalar(dd768[:], dd[:], float(1.0 / 768.0), None, ALU.mult)

            t1 = T63("p_t1")
            nc.vector.tensor_scalar(t1[:], alo[:], LO_U, -384.0, ALU.subtract, ALU.mult)
            c1 = T63("p_c1")
            nc.vector.tensor_tensor(c1[:], t1[:], inv[:], ALU.mult)
            nc.vector.tensor_scalar(c1[:], c1[:], 0.0, 384.0, ALU.max, ALU.min)
            nc.vector.tensor_scalar(c1[:], c1[:], MAGIC, MAGIC, ALU.add, ALU.subtract)
            lo_alo = T63("p_loalo")
            nc.vector.tensor_scalar(lo_alo[:], alo[:], LO_U, -1.0, ALU.subtract, ALU.mult)
            u1 = T63("p_u1")
            nc.vector.tensor_tensor(u1[:], dd768[:], c1[:], ALU.mult)
            nc.vector.tensor_tensor(u1[:], lo_alo[:], u1[:], ALU.subtract)
            s1c = T63("p_s1c")
            nc.vector.tensor_tensor(s1c[:], c1[:], u1[:], ALU.mult)

            t2 = T63("p_t2")
            nc.vector.tensor_scalar(t2[:], ahi[:], HI_U, 384.0, ALU.subtract, ALU.mult)
            c2 = T63("p_c2")
            nc.vector.tensor_tensor(c2[:], t2[:], inv[:], ALU.mult)
            nc.vector.tensor_scalar(c2[:], c2[:], 0.0, 384.0, ALU.max, ALU.min)
            nc.vector.tensor_scalar(c2[:], c2[:], MAGIC, MAGIC, ALU.add, ALU.subtract)
            ahi_hi = T63("p_ahihi")
            nc.vector.tensor_scalar(ahi_hi[:], ahi[:], HI_U, None, ALU.subtract)
            u2 = T63("p_u2")
            nc.vector.tensor_tensor(u2[:], dd768[:], c2[:], ALU.mult)
            nc.vector.tensor_tensor(u2[:], ahi_hi[:], u2[:], ALU.subtract)
            s2c = T63("p_s2c")
            nc.vector.tensor_tensor(s2c[:], c2[:], u2[:], ALU.mult)

            tall = apool.tile([128, 64], F32, tag="p_tall")
            slin = T63("p_slin")
            nc.vector.tensor_tensor(slin[:], a, b_, ALU.add)
            nc.vector.tensor_scalar(slin[:], slin[:], 192.0, None, ALU.mult)
            nc.vector.tensor_tensor(tall[:, 1:64], slin[:], s1c[:], ALU.add)
            nc.vector.tensor_tensor(tall[:, 1:64], tall[:, 1:64], s2c[:], ALU.subtract)
            nc.vector.tensor_scalar(tall[:, 0:1], f_ex[:, 0:1], LO_U, HI_U, ALU.max, ALU.min)
            nc.vector.tensor_scalar(tall[:, 0:1], tall[:, 0:1], 192.0, None, ALU.mult)
            trnd = apool.tile([128, 64], F32, tag="p_trnd")
            nc.vector.tensor_scalar(trnd[:], tall[:], MAGIC, MAGIC, ALU.add, ALU.subtract)
            nc.vector.tensor_tensor(tall[:], tall[:], trnd[:], ALU.subtract)
            cinc = apool.tile([128, 64], F32, tag="p_cinc")
            nc.vector.tensor_tensor_scan(cinc[:], tall[:], tall[:], 0.0, ALU.add, ALU.bypass)
            cu = apool.tile([128, NUNITS], F32, tag=f"cu{ex}")
            nc.gpsimd.memset(cu[:, 0:1], 0.0)
            nc.vector.tensor_copy(cu[:, 1:65], cinc[:])

            flo_u.append(flo)
            df_u.append(dfu)
            c_u.append(cu)
            l2_u.append(l2t)

        # ================= osc bank + noise branch, interleaved =================
        # Engines execute their queues in order, so emission order controls
        # overlap: alternate DVE-heavy osc groups with PE-heavy noise chunks.
        two_pi = float(2.0 * np.pi)
        groups = _osc_groups()
        lerp_state = [0]
        hm_state = [0]
        osc_done = [0, 0]
        hm_big = [apool.tile([128, 17, SEG], F16, tag=f"hmb{e}", name=f"hmb{e}")
                  for e in range(BPC)]

        def emit_osc_group(ex, g):
            glist, _ = groups[g]
            pm4 = ps_osc.tile([128, SEG], F32, tag="pm4")
            ph4 = opool.tile([128, 4 * SEG], F32, tag="ph4", bufs=2)
            s16 = opool.tile([128, 4 * SEG], F16, tag="s16", bufs=2)
            offs = []
            cur = 0
            for (j, c0, wdt) in glist:
                offs.append(cur)
                cur += wdt
            used = cur
            for gi, (j, c0, wdt) in enumerate(glist):
                fu = opool.tile([128, SEG], F32, tag="fu", bufs=4)
                eng = nc.vector if lerp_state[0] % 5 < 2 else nc.gpsimd
                eng.tensor_scalar(fu[:, 0:wdt], wt_t[:, 0:wdt],
                                  df_u[ex][:, j:j + 1],
                                  flo_u[ex][:, j:j + 1],
                                  ALU.mult, ALU.add)
                lerp_state[0] += 1
                nc.vector._custom_dve(
                    osc_op, out=ph4[:, offs[gi]:offs[gi] + wdt], in0=fu[:, 0:wdt],
                    in1=c_u[ex][:, j:j + 1].to_broadcast((128, wdt)),
                    s0=LO_U, s1=HI_U, imm2=MAGIC)
            nc.scalar.activation(s16[:, 0:used], ph4[:, 0:used], AF.Sin,
                                 bias=0.0, scale=two_pi)
            for gi, (j, c0, wdt) in enumerate(glist):
                nc.tensor.matmul(pm4[32 * gi:32 * gi + 2, 0:wdt],
                                 l2_u[ex][:, j, :],
                                 s16[:, offs[gi]:offs[gi] + wdt],
                                 start=True, stop=True,
                                 tile_position=(0, 32 * gi))
            wdma = glist[0][2]
            hmb = hm_big[ex]
            if hm_state[0] % 3 == 2:
                nc.vector.tensor_copy(hmb[:, g, 0:wdma], pm4[:, 0:wdma])
            else:
                nc.scalar.copy(hmb[:, g, 0:wdma], pm4[:, 0:wdma])
            hm_state[0] += 1
            osc_done[ex] += 1
            if osc_done[ex] % 4 == 0 or osc_done[ex] == len(groups):
                g1 = osc_done[ex]
                g0 = (g1 - 1) // 4 * 4
                nc.sync.dma_start(
                    h_out[ex, :, g0 * SEG:g1 * SEG],
                    hmb[:, g0:g1, :].rearrange("c g s -> c (g s)"))

        def noise_gen():
            # input pairs from h4 (fp32 -> fp8)
            hp = []
            for q in range(2):
                t8 = apool.tile([128, 2, BPC, 68], F8, tag=f"hp{q}", name=f"hp{q}")
                for s in range(2):
                    nc.vector.tensor_copy(t8[:, s], h4[2 * q + s][:])
                hp.append(t8)

            ycur = hp          # pair tiles, fp8
            TI = T0
            for li in range(4):
                TOUT = TI * 2
                WIDO = TOUT + 4
                last = li == 3
                odt = F16 if last else F8
                tagp = "yA" if li % 2 == 0 else "yB"
                ynxt = [apool.tile([128, 2, BPC, WIDO], odt, tag=f"{tagp}{q}",
                                   name=f"{tagp}{q}") for q in range(2)]
                for q in range(2):
                    nc.gpsimd.memset(ynxt[q][:, :, :, 0:2], 0.0)
                    nc.gpsimd.memset(ynxt[q][:, :, :, WIDO - 2:WIDO], 0.0)
                per_ex = BPC * TI > 512
                for eo in range(2):
                    wgt = w2pool.tile([128, 4, 2, 2, 512], F8, tag="wn8")
                    nc.sync.dma_start(wgt[:].bitcast(U8)
                                      .rearrange("c a q s o -> c (a q s o)"),
                                      wn8[li][eo, :, :])
                    for m in range(4):
                        bias_ap = bnl_t[:, 4 * li + m:4 * li + m + 1]
                        ex_sets = list(range(BPC)) if per_ex else [None]
                        for ex in ex_sets:
                            ncols = TI if per_ex else BPC * TI
                            pm = ps_mm.tile([128, 512], F32, tag="pconv")
                            i_mm = 0
                            for q in range(2):
                                for tap in range(4):
                                    off = tap + eo
                                    if per_ex:
                                        rhs = ycur[q][:, :, ex, off:off + TI]
                                    else:
                                        rhs = ycur[q][:, :, :, off:off + TI]
                                    lhsT = wgt[:, tap, q, :, 128 * m:128 * (m + 1)]
                                    nc.tensor.matmul(pm[:, 0:ncols], lhsT, rhs,
                                                     start=(i_mm == 0), stop=(i_mm == 7),
                                                     perf_mode=DR)
                                    i_mm += 1
                            if per_ex:
                                dst = ynxt[m // 2][:, m % 2, ex, 2 + eo:2 + eo + 2 * TI:2]
                                srcp = pm[:, 0:ncols]
                            else:
                                dst = ynxt[m // 2][:, m % 2, :, 2 + eo:2 + eo + 2 * TI:2]
                                srcp = pm[:, 0:ncols].rearrange("c (b t) -> c b t", b=BPC)
                            nc.scalar.activation(dst, srcp, AF.Prelu,
                                                 bias=bias_ap, scale=1.0, alpha=0.2)
                    yield
                ycur = ynxt
                TI = TOUT

            wh_t = w2pool.tile([128, 4, 34], F16, tag="wnh")
            for k in range(4):
                nc.sync.dma_start(wh_t[:, k, :], wnh[128 * k:128 * (k + 1), :])
            nl_sb = []
            for ex in range(BPC):
                nlt = apool.tile([34, FRAMES], F32, tag=f"nl{ex}")
                for half in range(2):
                    pm = ps_fft.tile([34, 512], F32, tag="pfft")
                    for k in range(4):
                        nc.tensor.matmul(pm[:],
                                         wh_t[:, k, :],
                                         ycur[k // 2][:, k % 2, ex,
                                                      2 + 512 * half:2 + 512 * (half + 1)],
                                         start=(k == 0), stop=(k == 3))
                    nc.scalar.activation(nlt[:, 512 * half:512 * (half + 1)], pm[:], AF.Square)
                nl_sb.append(nlt)
            yield

            # ---- noise FFT ----
            for ex in range(BPC):
                nzt = fpool.tile([WIN, FRAMES], F32, tag="nz")
                nc.sync.dma_start_transpose(nzt[:], noi[ex, :, :])
                nz2 = fpool.tile([WIN, FRAMES], F32R, tag="nz2")
                nc.vector.tensor_scalar(nz2[:], nzt[:], 2.0, -1.0, ALU.mult, ALU.add)
                fcs = fpool.tile([34, FRAMES], F32R, tag="fcs")
                for half in range(2):
                    pm = ps_fft.tile([34, 512], F32, tag="pfft")
                    nc.tensor.matmul(pm[:], fcat_t[:],
                                     nz2[:, 512 * half:512 * (half + 1)],
                                     start=True, stop=True)
                    nc.vector.tensor_tensor(fcs[:, 512 * half:512 * (half + 1)], pm[:],
                                            nl_sb[ex][:, 512 * half:512 * (half + 1)],
                                            ALU.mult)
                frsA = fpool.tile([16, FRAMES], F32, tag="frsA")
                frsB = fpool.tile([16, FRAMES], F32, tag="frsB")
                for half in range(2):
                    pm = ps_fft.tile([34, 512], F32, tag="pfft")
                    nc.tensor.matmul(pm[0:16, :], gmat_t[:, 0:16],
                                     fcs[:, 512 * half:512 * (half + 1)],
                                     start=True, stop=True)
                    nc.scalar.copy(frsA[:, 512 * half:512 * (half + 1)], pm[0:16, :])
                    pm2 = ps_fft.tile([34, 512], F32, tag="pfft")
                    nc.tensor.matmul(pm2[0:16, :], gmat_t[:, 16:32],
                                     fcs[:, 512 * half:512 * (half + 1)],
                                     start=True, stop=True)
                    nc.scalar.copy(frsB[:, 512 * half:512 * (half + 1)], pm2[0:16, :])
                nsb = fpool.tile([16, FRAMES], F32, tag="nsb")
                nc.vector.tensor_copy(nsb[:, 0:1], frsA[:, 0:1])
                nc.vector.tensor_tensor(nsb[:, 1:FRAMES], frsA[:, 1:FRAMES],
                                        frsB[:, 0:FRAMES - 1], ALU.add)
                nc.sync.dma_start(n_out[16 * ex:16 * (ex + 1), :], nsb[:])
                yield

        # round-robin: 3 osc groups per noise chunk, then drain the rest
        osc_list = [(ex, g) for ex in range(BPC) for g in range(len(groups))]
        osc_i = 0
        for _ in noise_gen():
            for _ in range(3):
                if osc_i < len(osc_list):
                    emit_osc_group(*osc_list[osc_i])
                    osc_i += 1
        while osc_i < len(osc_list):
            emit_osc_group(*osc_list[osc_i])
            osc_i += 1

    nc.compile()
    _BUILD_CACHE["nc"] = nc
    return nc


_W_PAT = None


def _wpat():
    global _W_PAT
    if _W_PAT is None:
        w = np.zeros(UP_LEN, np.float32)
        kk = ((np.arange(SEG) + 0.5) / SEG).astype(np.float32)
        for s in range(NSEG):
            w[EDGE + SEG * s: EDGE + SEG * (s + 1)] = kk
        _W_PAT = w
    return _W_PAT


def _prep_shared(inputs):
    d = {}

    def hilo(key, arr):
        a = np.ascontiguousarray(arr).astype(np.float32)
        h = a.astype(np.float16)
        l = (a - h.astype(np.float32)).astype(np.float16)
        d[key + "h"] = np.ascontiguousarray(h)
        d[key + "l"] = np.ascontiguousarray(l)

    hilo("wm0", inputs["w_main0"][:, :, 0].T)
    for i in (1, 2, 3):
        hilo(f"wm{i}", np.asarray(inputs[f"w_main{i}"]).transpose(1, 2, 0))
    hilo("wfq", inputs["w_freq"][:, :, 0].T)
    for l in range(4):
        W = np.asarray(inputs[f"w_nl{l}"])
        We = np.stack([W[:, :, 0], W[:, :, 1] + W[:, :, 2],
                       W[:, :, 3] + W[:, :, 4], W[:, :, 5] + W[:, :, 6]], -1)
        Wo = np.stack([W[:, :, 0] + W[:, :, 1], W[:, :, 2] + W[:, :, 3],
                       W[:, :, 4] + W[:, :, 5], W[:, :, 6]], -1)
        arr = np.stack([We.transpose(1, 2, 0), Wo.transpose(1, 2, 0)], 0)
        # arr: [2eo, cin512, tap4, cout512] -> [2, p128, tap, q2, s2, cout]
        arr = arr.reshape(2, 2, 2, 128, 4, 512)          # [eo, q, s, p, tap, co]
        arr = arr.transpose(0, 3, 4, 1, 2, 5)            # [eo, p, tap, q, s, co]
        a8 = np.ascontiguousarray(arr).astype(ml_dtypes.float8_e4m3)
        d[f"wn8_{l}"] = np.ascontiguousarray(
            a8.reshape(2, 128, 4 * 2 * 2 * 512).view(np.uint8))
    wh = np.asarray(inputs["w_noise_loud"])[:, :, 0].T          # [512, 17]
    d["wnh"] = np.ascontiguousarray(
        np.concatenate([wh, wh], 1).astype(np.float16))
    bn = np.zeros((128, 16), np.float32)
    for l in range(4):
        bl = np.asarray(inputs[f"b_nl{l}"]).reshape(4, 128)
        for m in range(4):
            bn[:, 4 * l + m] = bl[m]
    d["bnl"] = bn
    d["wt"] = np.ascontiguousarray(
        np.broadcast_to(((np.arange(SEG) + 0.5) / SEG).astype(np.float32), (128, SEG)))
    k = np.arange(WIN)[:, None].astype(np.float64)
    j = np.arange(17)[None, :].astype(np.float64)
    fre = np.cos(-2 * np.pi * k * j / WIN) / np.sqrt(WIN)
    fim = np.sin(-2 * np.pi * k * j / WIN) / np.sqrt(WIN)
    d["fcat"] = np.ascontiguousarray(np.concatenate([fre, fim], 1).astype(np.float32))
    t = np.arange(WIN)[None, :].astype(np.float64)
    jj = np.arange(17)[:, None].astype(np.float64)
    wgt = np.where((jj == 0) | (jj == 16), 1.0, 2.0)
    gre = wgt * np.cos(2 * np.pi * jj * t / WIN) / np.sqrt(WIN)
    gim = -wgt * np.sin(2 * np.pi * jj * t / WIN) / np.sqrt(WIN)
    d["gmat"] = np.ascontiguousarray(np.concatenate([gre, gim], 0).astype(np.float32))
    d["cesc"] = (0.5 * _ERBS / SR).astype(np.float32).reshape(128, 1)
    d["cebi"] = (_CENTERS / SR).astype(np.float32).reshape(128, 1)
    return d


def _in_maps(inputs):
    shared = _prep_shared(inputs)
    x = np.asarray(inputs["x"], np.float32)
    noise = np.asarray(inputs["noise"], np.float32)
    maps = []
    for c in range(NCORES):
        m = dict(shared)
        xc = x[BPC * c:BPC * (c + 1)]
        xh = xc.astype(np.float16)
        m["x3h"] = np.ascontiguousarray(xh)
        m["x3l"] = np.ascontiguousarray((xc - xh.astype(np.float32)).astype(np.float16))
        m["noi"] = np.ascontiguousarray(noise[BPC * c:BPC * (c + 1)])
        maps.append(m)
    return maps


def _assemble(results):
    wpat = _wpat()
    groups = _osc_groups()
    out = np.empty((B, 1, TOTAL), np.float32)
    for c in range(NCORES):
        h_o = results[c]["h_out"]       # [BPC, 128, 17*SEG] fp16
        n_o = results[c]["n_out"]
        for ex in range(BPC):
            bidx = BPC * c + ex
            hb = h_o[ex].reshape(128, 17, SEG).astype(np.float32)
            lo = np.empty(UP_LEN, np.float32)
            dl = np.empty(UP_LEN, np.float32)
            for g, (glist, is_edge) in enumerate(groups):
                for gi, (j, c0, wdt) in enumerate(glist):
                    lo[c0:c0 + wdt] = hb[32 * gi, g, 0:wdt]
                    dl[c0:c0 + wdt] = hb[32 * gi + 1, g, 0:wdt]
            sig = lo + wpat * dl
            nzf = np.ascontiguousarray(n_o[16 * ex:16 * (ex + 1)].T).reshape(TOTAL)
            sig[CROP:CROP + TOTAL] += nzf
            mx = np.abs(sig).max()
            out[bidx, 0] = sig[CROP:CROP + TOTAL] / (mx + np.float32(1e-8))
    return out


def kernel(**inputs) -> np.ndarray:
    nc = _build_program()
    maps = _in_maps(inputs)
    res = bass_utils.run_bass_kernel_spmd(nc, maps, core_ids=list(range(NCORES)))
    return _assemble([res.results[c] for c in range(NCORES)])


# revision 19
# speedup vs baseline: 1.0035x; 1.0035x over previous
"""DDSP generator Bass kernel for Trainium2, 8-core data parallel.

Sharding: batch 16 -> 8 cores x 2 examples each. Weights replicated.
Per core:
  stage1: main conv stack (fp32 PE) -> h; osc head -> l (amp^2), f (Hz/SR)
  osc bank, per 384-sample resize segment (plus two 192 edge segments):
      lerp (DVE/Pool tensor_scalar, per-partition scale/bias) ->
      custom DVE op (clip + cumsum + wrap to [-0.5, 0.5] cycles, one pass) ->
      ACT Sin (per group of 4 units) -> fp16 -> m=2 PE reduce matmul with
      lhsT = [l_lo | dl], 4 segments packed per PSUM bank via tile_position ->
      direct PSUM -> DRAM DMA (partition-strided, one per group).
  noise branch: 4x (2x-upsample conv k7) via even/odd stride trick
      (host-combined 4-tap weights), fp8e4m3 DoubleRow matmuls (0.5 cyc/row,
      contraction 256); activations quantized to fp8 between layers; last
      layer kept fp16 for the head conv (duplicated 34-col weights) +
      Square -> n_l on partitions 0..33.
  noise FFT: rfft/irfft as fp32r DFT matmuls, filter on DVE, overlap-add.
Host: recombine the two reduce rows with the lerp-weight pattern, pad,
      add noise, normalize, crop (O(output) numpy work only).
"""

import numpy as np
from contextlib import ExitStack

import ml_dtypes

import concourse.bass as bass
import concourse.tile as tile
from concourse import bacc, mybir
from concourse import bass_utils
from concourse import dve_ops
from concourse.dve_spec import Spec, Src0, Src1, C0, C1, C2, scan, minn, maxx, AluOp, lower
from concourse.dve_uop import DveOpSpec

F32 = mybir.dt.float32
F16 = mybir.dt.float16
F8 = mybir.dt.float8e4
U8 = mybir.dt.uint8
F32R = mybir.dt.float32r
AF = mybir.ActivationFunctionType
ALU = mybir.AluOpType
DR = mybir.MatmulPerfMode.DoubleRow

SR = 11025.0
UP_LEN = 24576
TOTAL = 16384
WIN = 32
FRAMES = 1024
CROP = 4096
B = 16
NCORES = 8
BPC = 2
T0 = 64
SEG = 384
NSEG = 63
EDGE = 192
NUNITS = NSEG + 2
LO_U = 20.0 / SR
HI_U = 0.5
MAGIC = 12582912.0

_CENTERS = np.geomspace(20.0, SR / 2.0 - 20.0, 128).astype(np.float32)
_ERBS = (_CENTERS * np.float32(0.108) + np.float32(24.7)).astype(np.float32)


def _osc_ref(in0, in1, s0, s1, imm2):
    v = np.minimum(np.maximum(in0, np.float32(s0)), np.float32(s1)).astype(np.float32)
    u = np.cumsum(v.astype(np.float64), axis=-1).astype(np.float32)
    y = (u + in1).astype(np.float32)
    r = ((y + np.float32(imm2)) - np.float32(imm2)).astype(np.float32)
    return (y - r).astype(np.float32)


def _register_osc_op():
    if hasattr(dve_ops, "CUSTOM_DVE_OPS_BY_NAME") and \
            "OSC_PHASE_ANT" in dve_ops.CUSTOM_DVE_OPS_BY_NAME:
        return dve_ops.CUSTOM_DVE_OPS_BY_NAME["OSC_PHASE_ANT"]
    body_v = minn(maxx(Src0, C0), C1)
    body_u = scan(AluOp.ADD, body_v)
    body_y = body_u + Src1
    body = body_y - ((body_y + C2) - C2)
    spec = Spec(body=body, reference=_osc_ref)
    sha = {}
    for ver in ("v3",):
        s = DveOpSpec(name="OSC_PHASE_ANT", opcode=1, uops=lower(spec, ver=ver),
                      rd1_en=True)
        sha[ver] = s.sha(ver)
    op = dve_ops.DveOp("OSC_PHASE_ANT", spec, subdim=False, uops_sha=sha)
    dve_ops.OPS.append(op)
    dve_ops.CUSTOM_DVE_SPECS[op.name] = op.spec
    dve_ops._SUB_OPCODE_FOR_NAME[op.name] = max(dve_ops._SUB_OPCODE_FOR_NAME.values()) + 1
    if not hasattr(dve_ops, "CUSTOM_DVE_OPS_BY_NAME"):
        dve_ops.CUSTOM_DVE_OPS_BY_NAME = {}
    dve_ops.CUSTOM_DVE_OPS_BY_NAME[op.name] = op
    return op


def _osc_groups():
    """Per-example unit grouping: 16 groups of SEG units (last has 3) plus
    one edge group [unit0, unit64]. Returns list of (glist, is_edge) where
    glist entries are (j, c0, wdt)."""
    groups = []
    for g in range(16):
        js = [1 + 4 * g + i for i in range(4) if 1 + 4 * g + i <= 63]
        groups.append(([(j, EDGE + SEG * (j - 1), SEG) for j in js], False))
    groups.append(([(0, 0, EDGE), (NUNITS - 1, UP_LEN - EDGE, EDGE)], True))
    return groups


_BUILD_CACHE = {}


def _build_program():
    if "nc" in _BUILD_CACHE:
        return _BUILD_CACHE["nc"]
    osc_op = _register_osc_op()

    nc = bacc.Bacc("TRN2", target_bir_lowering=False, debug=False, num_devices=1)

    dI = lambda n, s, dt=F32: nc.dram_tensor(n, s, dt, kind="ExternalInput").ap()
    dO = lambda n, s, dt=F32: nc.dram_tensor(n, s, dt, kind="ExternalOutput").ap()

    x3h = dI("x3h", [BPC, 256, T0], F16)
    x3l = dI("x3l", [BPC, 256, T0], F16)
    noi = dI("noi", [BPC, FRAMES, WIN])
    wm0 = [dI(f"wm0{s}", [256, 512], F16) for s in "hl"]
    wmL = [[dI(f"wm{i}{s}", [512, 3, 512], F16) for s in "hl"] for i in (1, 2, 3)]
    wfq = [dI(f"wfq{s}", [512, 256], F16) for s in "hl"]
    wn8 = [dI(f"wn8_{l}", [2, 128, 4 * 2 * 2 * 512], U8) for l in range(4)]
    wnh = dI("wnh", [512, 34], F16)                       # head, duplicated cols
    bnl = dI("bnl", [128, 16])
    wt = dI("wt", [128, SEG])
    fcat = dI("fcat", [WIN, 34])
    gmat = dI("gmat", [34, WIN])
    cesc = dI("cesc", [128, 1])
    cebi = dI("cebi", [128, 1])

    h_out = dO("h_out", [BPC, 128, 17 * SEG], F16)
    n_out = dO("n_out", [16 * BPC, FRAMES])

    with tile.TileContext(nc) as tc, ExitStack() as ctx:
        cpool = ctx.enter_context(tc.tile_pool(name="consts", bufs=1))
        apool = ctx.enter_context(tc.tile_pool(name="acts", bufs=1))
        fpool = ctx.enter_context(tc.tile_pool(name="fft", bufs=1))
        opool = ctx.enter_context(tc.tile_pool(name="osc", bufs=1))
        w1pool = ctx.enter_context(tc.tile_pool(name="w1", bufs=2))
        w2pool = ctx.enter_context(tc.tile_pool(name="w2", bufs=3))
        ps_mm = ctx.enter_context(tc.tile_pool(name="psmm", bufs=4, space="PSUM"))
        ps_osc = ctx.enter_context(tc.tile_pool(name="psosc", bufs=2, space="PSUM"))
        ps_fft = ctx.enter_context(tc.tile_pool(name="psfft", bufs=2, space="PSUM"))

        wt_t = cpool.tile([128, SEG], F32)
        nc.sync.dma_start(wt_t[:], wt[:])
        cesc_t = cpool.tile([128, 1], F32)
        nc.sync.dma_start(cesc_t[:], cesc[:])
        cebi_t = cpool.tile([128, 1], F32)
        nc.sync.dma_start(cebi_t[:], cebi[:])
        fcat_t = cpool.tile([WIN, 34], F32R)
        nc.sync.dma_start(fcat_t[:], fcat[:].bitcast(F32R))
        gmat_t = cpool.tile([34, WIN], F32R)
        nc.sync.dma_start(gmat_t[:], gmat[:].bitcast(F32R))
        bnl_t = cpool.tile([128, 16], F32)
        nc.sync.dma_start(bnl_t[:], bnl[:])

        # ================= stage 1: main conv stack =================
        x_t = []
        for k in range(2):
            xth = apool.tile([128, BPC, T0], F16, tag=f"xh{k}")
            nc.sync.dma_start(xth[:], x3h[:, 128 * k:128 * (k + 1), :].rearrange("b c t -> c b t"))
            xtl = apool.tile([128, BPC, T0], F16, tag=f"xl{k}")
            nc.sync.dma_start(xtl[:], x3l[:, 128 * k:128 * (k + 1), :].rearrange("b c t -> c b t"))
            x_t.append((xth, xtl))

        wm0_t = []
        for k in range(2):
            wh = w1pool.tile([128, 512], F16, tag=f"wm0h_{k}")
            nc.sync.dma_start(wh[:], wm0[0][128 * k:128 * (k + 1), :])
            wl_ = w1pool.tile([128, 512], F16, tag=f"wm0l_{k}")
            nc.sync.dma_start(wl_[:], wm0[1][128 * k:128 * (k + 1), :])
            wm0_t.append((wh, wl_))

        NCOL = BPC * T0
        h1 = []
        for m in range(4):
            pm = ps_mm.tile([128, 512], F32, tag="pconv")
            i_mm = 0
            for k in range(2):
                ms = slice(128 * m, 128 * (m + 1))
                for lh, rh in ((0, 0), (0, 1), (1, 0)):
                    nc.tensor.matmul(pm[:, 0:NCOL], wm0_t[k][lh][:, ms],
                                     x_t[k][rh][:],
                                     start=(i_mm == 0), stop=(i_mm == 5))
                    i_mm += 1
            ht = apool.tile([128, BPC, 66], F32, tag=f"hA{m}")
            nc.gpsimd.memset(ht[:, :, 0:1], 0.0)
            nc.gpsimd.memset(ht[:, :, 65:66], 0.0)
            nc.scalar.activation(ht[:, :, 1:65],
                                 pm[:, 0:NCOL].rearrange("c (b t) -> c b t", b=BPC),
                                 AF.Prelu, bias=0.0, scale=1.0, alpha=0.2)
            h1.append(ht)

        def split16(tiles, PAD, WID, tagp):
            # fp32 h tiles -> (hi, lo) fp16 pairs, pads included (zero)
            out = []
            for k, ht in enumerate(tiles):
                hh = apool.tile([128, BPC, WID], F16, tag=f"{tagp}h{k}")
                nc.vector.tensor_copy(hh[:], ht[:])
                hl = apool.tile([128, BPC, WID], F16, tag=f"{tagp}l{k}")
                nc.vector.tensor_tensor(hl[:], ht[:], hh[:], ALU.subtract)
                out.append((hh, hl))
            return out

        hcur = split16(h1, 1, 66, "sA")
        for li in range(3):
            wl = []
            for k in range(4):
                wh = w1pool.tile([128, 3 * 512], F16, tag=f"wmLh_{k}")
                nc.sync.dma_start(wh[:], wmL[li][0][128 * k:128 * (k + 1), :, :]
                                  .rearrange("c a o -> c (a o)"))
                wlo = w1pool.tile([128, 3 * 512], F16, tag=f"wmLl_{k}")
                nc.sync.dma_start(wlo[:], wmL[li][1][128 * k:128 * (k + 1), :, :]
                                  .rearrange("c a o -> c (a o)"))
                wl.append((wh, wlo))
            last = li == 2
            PAD = 2 if last else 1
            WID = T0 + 2 * PAD
            tagp = "hB" if li % 2 == 0 else "hA"
            hnxt = []
            for m in range(4):
                pm = ps_mm.tile([128, 512], F32, tag="pconv")
                i_mm = 0
                for k in range(4):
                    for tap in range(3):
                        wsl = slice(512 * tap + 128 * m, 512 * tap + 128 * (m + 1))
                        for lh, rh in ((0, 0), (0, 1), (1, 0)):
                            nc.tensor.matmul(
                                pm[:, 0:NCOL],
                                wl[k][lh][:, wsl],
                                hcur[k][rh][:, :, tap:tap + T0],
                                start=(i_mm == 0), stop=(i_mm == 35))
                            i_mm += 1
                ht = apool.tile([128, BPC, WID], F32,
                                tag=(f"h4_{m}" if last else f"{tagp}{m}"))
                nc.gpsimd.memset(ht[:, :, 0:PAD], 0.0)
                nc.gpsimd.memset(ht[:, :, PAD + T0:WID], 0.0)
                nc.scalar.activation(ht[:, :, PAD:PAD + T0],
                                     pm[:, 0:NCOL].rearrange("c (b t) -> c b t", b=BPC),
                                     AF.Prelu, bias=0.0, scale=1.0, alpha=0.2)
                hnxt.append(ht)
            hcur = split16(hnxt, PAD, WID, "sB" if li % 2 == 0 else "sA")
            if last:
                h4 = hnxt   # 4 x [128, BPC, 68] fp32, pad 2
        h4s = hcur

        wfq_t = []
        for k in range(4):
            wh = w1pool.tile([128, 256], F16, tag=f"wfqh{k}")
            nc.sync.dma_start(wh[:], wfq[0][128 * k:128 * (k + 1), :])
            wlo = w1pool.tile([128, 256], F16, tag=f"wfql{k}")
            nc.sync.dma_start(wlo[:], wfq[1][128 * k:128 * (k + 1), :])
            wfq_t.append((wh, wlo))
        l_sb = apool.tile([128, BPC, T0], F32, tag="l_sb")
        f_sb = apool.tile([128, BPC, T0], F32, tag="f_sb")
        for m in range(2):
            pm = ps_mm.tile([128, 512], F32, tag="pconv")
            i_mm = 0
            for k in range(4):
                ms = slice(128 * m, 128 * (m + 1))
                for lh, rh in ((0, 0), (0, 1), (1, 0)):
                    nc.tensor.matmul(pm[:, 0:NCOL], wfq_t[k][lh][:, ms],
                                     h4s[k][rh][:, :, 2:2 + T0],
                                     start=(i_mm == 0), stop=(i_mm == 11))
                    i_mm += 1
            if m == 0:
                nc.scalar.activation(l_sb[:],
                                     pm[:, 0:NCOL].rearrange("c (b t) -> c b t", b=BPC),
                                     AF.Square)
            else:
                tanh_t = apool.tile([128, BPC, T0], F32, tag="tanh")
                nc.scalar.activation(tanh_t[:],
                                     pm[:, 0:NCOL].rearrange("c (b t) -> c b t", b=BPC),
                                     AF.Tanh)
                nc.scalar.activation(f_sb[:], tanh_t[:],
                                     AF.Identity, bias=cebi_t[:], scale=cesc_t[:])

        # ================= osc prep =================
        flo_u, df_u, c_u, l2_u = [], [], [], []
        for ex in range(BPC):
            f_ex = f_sb[:, ex, :]
            l_ex = l_sb[:, ex, :]

            flo = apool.tile([128, NUNITS], F32, tag=f"flo{ex}")
            nc.vector.tensor_copy(flo[:, 0:1], f_ex[:, 0:1])
            nc.vector.tensor_copy(flo[:, 1:65], f_ex[:, 0:64])
            dfu = apool.tile([128, NUNITS], F32, tag=f"dfu{ex}")
            nc.gpsimd.memset(dfu[:, 0:1], 0.0)
            nc.gpsimd.memset(dfu[:, 64:65], 0.0)
            nc.gpsimd.tensor_tensor(dfu[:, 1:64], f_ex[:, 1:64], f_ex[:, 0:63], ALU.subtract)

            l2t = apool.tile([128, NUNITS, 2], F16, tag=f"l2{ex}")
            nc.vector.tensor_copy(l2t[:, 0:1, 0], l_ex[:, 0:1])
            nc.vector.tensor_copy(l2t[:, 1:65, 0], l_ex[:, 0:64])
            nc.gpsimd.memset(l2t[:, 0:1, 1], 0.0)
            nc.gpsimd.memset(l2t[:, 64:65, 1], 0.0)
            nc.gpsimd.tensor_tensor(l2t[:, 1:64, 1], l_ex[:, 1:64], l_ex[:, 0:63], ALU.subtract)

            a = f_ex[:, 0:63]
            b_ = f_ex[:, 1:64]

            def T63(tag):
                return apool.tile([128, 63], F32, tag=tag, name=tag)

            alo = T63("p_alo")
            nc.vector.tensor_tensor(alo[:], a, b_, ALU.min)
            ahi = T63("p_ahi")
            nc.vector.tensor_tensor(ahi[:], a, b_, ALU.max)
            dd = T63("p_dd")
            nc.vector.tensor_tensor(dd[:], ahi[:], alo[:], ALU.subtract)
            ddc = T63("p_ddc")
            nc.vector.tensor_scalar(ddc[:], dd[:], 1e-30, None, ALU.max)
            inv = T63("p_inv")
            nc.vector.reciprocal(inv[:], ddc[:])
            dd768 = T63("p_dd768")
            nc.vector.tensor_scalar(dd768[:], dd[:], float(1.0 / 768.0), None, ALU.mult)

            t1 = T63("p_t1")
            nc.vector.tensor_scalar(t1[:], alo[:], LO_U, -384.0, ALU.subtract, ALU.mult)
            c1 = T63("p_c1")
            nc.vector.tensor_tensor(c1[:], t1[:], inv[:], ALU.mult)
            nc.vector.tensor_scalar(c1[:], c1[:], 0.0, 384.0, ALU.max, ALU.min)
            nc.vector.tensor_scalar(c1[:], c1[:], MAGIC, MAGIC, ALU.add, ALU.subtract)
            lo_alo = T63("p_loalo")
            nc.vector.tensor_scalar(lo_alo[:], alo[:], LO_U, -1.0, ALU.subtract, ALU.mult)
            u1 = T63("p_u1")
            nc.vector.tensor_tensor(u1[:], dd768[:], c1[:], ALU.mult)
            nc.vector.tensor_tensor(u1[:], lo_alo[:], u1[:], ALU.subtract)
            s1c = T63("p_s1c")
            nc.vector.tensor_tensor(s1c[:], c1[:], u1[:], ALU.mult)

            t2 = T63("p_t2")
            nc.vector.tensor_scalar(t2[:], ahi[:], HI_U, 384.0, ALU.subtract, ALU.mult)
            c2 = T63("p_c2")
            nc.vector.tensor_tensor(c2[:], t2[:], inv[:], ALU.mult)
            nc.vector.tensor_scalar(c2[:], c2[:], 0.0, 384.0, ALU.max, ALU.min)
            nc.vector.tensor_scalar(c2[:], c2[:], MAGIC, MAGIC, ALU.add, ALU.subtract)
            ahi_hi = T63("p_ahihi")
            nc.vector.tensor_scalar(ahi_hi[:], ahi[:], HI_U, None, ALU.subtract)
            u2 = T63("p_u2")
            nc.vector.tensor_tensor(u2[:], dd768[:], c2[:], ALU.mult)
            nc.vector.tensor_tensor(u2[:], ahi_hi[:], u2[:], ALU.subtract)
            s2c = T63("p_s2c")
            nc.vector.tensor_tensor(s2c[:], c2[:], u2[:], ALU.mult)

            tall = apool.tile([128, 64], F32, tag="p_tall")
            slin = T63("p_slin")
            nc.vector.tensor_tensor(slin[:], a, b_, ALU.add)
            nc.vector.tensor_scalar(slin[:], slin[:], 192.0, None, ALU.mult)
            nc.vector.tensor_tensor(tall[:, 1:64], slin[:], s1c[:], ALU.add)
            nc.vector.tensor_tensor(tall[:, 1:64], tall[:, 1:64], s2c[:], ALU.subtract)
            nc.vector.tensor_scalar(tall[:, 0:1], f_ex[:, 0:1], LO_U, HI_U, ALU.max, ALU.min)
            nc.vector.tensor_scalar(tall[:, 0:1], tall[:, 0:1], 192.0, None, ALU.mult)
            trnd = apool.tile([128, 64], F32, tag="p_trnd")
            nc.vector.tensor_scalar(trnd[:], tall[:], MAGIC, MAGIC, ALU.add, ALU.subtract)
            nc.vector.tensor_tensor(tall[:], tall[:], trnd[:], ALU.subtract)
            cinc = apool.tile([128, 64], F32, tag="p_cinc")
            nc.vector.tensor_tensor_scan(cinc[:], tall[:], tall[:], 0.0, ALU.add, ALU.bypass)
            cu = apool.tile([128, NUNITS], F32, tag=f"cu{ex}")
            nc.gpsimd.memset(cu[:, 0:1], 0.0)
            nc.vector.tensor_copy(cu[:, 1:65], cinc[:])

            flo_u.append(flo)
            df_u.append(dfu)
            c_u.append(cu)
            l2_u.append(l2t)

        # ================= osc bank + noise branch, interleaved =================
        # Engines execute their queues in order, so emission order controls
        # overlap: alternate DVE-heavy osc groups with PE-heavy noise chunks.
        two_pi = float(2.0 * np.pi)
        groups = _osc_groups()
        lerp_state = [0]
        hm_state = [0]
        osc_done = [0, 0]
        hm_chunk = [None]

        def emit_osc_group(ex, g):
            glist, _ = groups[g]
            pm4 = ps_osc.tile([128, SEG], F32, tag="pm4")
            ph4 = opool.tile([128, 4 * SEG], F32, tag="ph4", bufs=2)
            s16 = opool.tile([128, 4 * SEG], F16, tag="s16", bufs=3)
            offs = []
            cur = 0
            for (j, c0, wdt) in glist:
                offs.append(cur)
                cur += wdt
            used = cur
            for gi, (j, c0, wdt) in enumerate(glist):
                fu = opool.tile([128, SEG], F32, tag="fu", bufs=4)
                eng = nc.vector if lerp_state[0] % 5 < 2 else nc.gpsimd
                eng.tensor_scalar(fu[:, 0:wdt], wt_t[:, 0:wdt],
                                  df_u[ex][:, j:j + 1],
                                  flo_u[ex][:, j:j + 1],
                                  ALU.mult, ALU.add)
                lerp_state[0] += 1
                nc.vector._custom_dve(
                    osc_op, out=ph4[:, offs[gi]:offs[gi] + wdt], in0=fu[:, 0:wdt],
                    in1=c_u[ex][:, j:j + 1].to_broadcast((128, wdt)),
                    s0=LO_U, s1=HI_U, imm2=MAGIC)
            nc.scalar.activation(s16[:, 0:used], ph4[:, 0:used], AF.Sin,
                                 bias=0.0, scale=two_pi)
            for gi, (j, c0, wdt) in enumerate(glist):
                nc.tensor.matmul(pm4[32 * gi:32 * gi + 2, 0:wdt],
                                 l2_u[ex][:, j, :],
                                 s16[:, offs[gi]:offs[gi] + wdt],
                                 start=True, stop=True,
                                 tile_position=(0, 32 * gi))
            wdma = glist[0][2]
            if g % 4 == 0:
                hm_chunk[0] = opool.tile([128, 4, SEG], F16, tag="hmb", bufs=3, name="hmb")
            hmb = hm_chunk[0]
            slot = g % 4
            if hm_state[0] % 3 == 2:
                nc.vector.tensor_copy(hmb[:, slot, 0:wdma], pm4[:, 0:wdma])
            else:
                nc.scalar.copy(hmb[:, slot, 0:wdma], pm4[:, 0:wdma])
            hm_state[0] += 1
            osc_done[ex] += 1
            if slot == 3 or g == len(groups) - 1:
                g0 = g - slot
                nc.sync.dma_start(
                    h_out[ex, :, g0 * SEG:(g + 1) * SEG],
                    hmb[:, 0:slot + 1, :].rearrange("c g s -> c (g s)"))

        def noise_gen():
            # input pairs from h4 (fp32 -> fp8)
            hp = []
            for q in range(2):
                t8 = apool.tile([128, 2, BPC, 68], F8, tag=f"hp{q}", name=f"hp{q}")
                for s in range(2):
                    nc.vector.tensor_copy(t8[:, s], h4[2 * q + s][:])
                hp.append(t8)

            ycur = hp          # pair tiles, fp8
            TI = T0
            for li in range(4):
                TOUT = TI * 2
                WIDO = TOUT + 4
                last = li == 3
                odt = F16 if last else F8
                tagp = "yA" if li % 2 == 0 else "yB"
                ynxt = [apool.tile([128, 2, BPC, WIDO], odt, tag=f"{tagp}{q}",
                                   name=f"{tagp}{q}") for q in range(2)]
                for q in range(2):
                    nc.gpsimd.memset(ynxt[q][:, :, :, 0:2], 0.0)
                    nc.gpsimd.memset(ynxt[q][:, :, :, WIDO - 2:WIDO], 0.0)
                per_ex = BPC * TI > 512
                for eo in range(2):
                    wgt = w2pool.tile([128, 4, 2, 2, 512], F8, tag="wn8")
                    nc.sync.dma_start(wgt[:].bitcast(U8)
                                      .rearrange("c a q s o -> c (a q s o)"),
                                      wn8[li][eo, :, :])
                    for m in range(4):
                        bias_ap = bnl_t[:, 4 * li + m:4 * li + m + 1]
                        ex_sets = list(range(BPC)) if per_ex else [None]
                        for ex in ex_sets:
                            ncols = TI if per_ex else BPC * TI
                            pm = ps_mm.tile([128, 512], F32, tag="pconv")
                            i_mm = 0
                            for q in range(2):
                                for tap in range(4):
                                    off = tap + eo
                                    if per_ex:
                                        rhs = ycur[q][:, :, ex, off:off + TI]
                                    else:
                                        rhs = ycur[q][:, :, :, off:off + TI]
                                    lhsT = wgt[:, tap, q, :, 128 * m:128 * (m + 1)]
                                    nc.tensor.matmul(pm[:, 0:ncols], lhsT, rhs,
                                                     start=(i_mm == 0), stop=(i_mm == 7),
                                                     perf_mode=DR)
                                    i_mm += 1
                            if per_ex:
                                dst = ynxt[m // 2][:, m % 2, ex, 2 + eo:2 + eo + 2 * TI:2]
                                srcp = pm[:, 0:ncols]
                            else:
                                dst = ynxt[m // 2][:, m % 2, :, 2 + eo:2 + eo + 2 * TI:2]
                                srcp = pm[:, 0:ncols].rearrange("c (b t) -> c b t", b=BPC)
                            nc.scalar.activation(dst, srcp, AF.Prelu,
                                                 bias=bias_ap, scale=1.0, alpha=0.2)
                    yield
                ycur = ynxt
                TI = TOUT

            wh_t = w2pool.tile([128, 4, 34], F16, tag="wnh")
            for k in range(4):
                nc.sync.dma_start(wh_t[:, k, :], wnh[128 * k:128 * (k + 1), :])
            nl_sb = []
            for ex in range(BPC):
                nlt = apool.tile([34, FRAMES], F32, tag=f"nl{ex}")
                for half in range(2):
                    pm = ps_fft.tile([34, 512], F32, tag="pfft")
                    for k in range(4):
                        nc.tensor.matmul(pm[:],
                                         wh_t[:, k, :],
                                         ycur[k // 2][:, k % 2, ex,
                                                      2 + 512 * half:2 + 512 * (half + 1)],
                                         start=(k == 0), stop=(k == 3))
                    nc.scalar.activation(nlt[:, 512 * half:512 * (half + 1)], pm[:], AF.Square)
                nl_sb.append(nlt)
            yield

            # ---- noise FFT ----
            for ex in range(BPC):
                nzt = fpool.tile([WIN, FRAMES], F32, tag="nz")
                nc.sync.dma_start_transpose(nzt[:], noi[ex, :, :])
                nz2 = fpool.tile([WIN, FRAMES], F32R, tag="nz2")
                nc.vector.tensor_scalar(nz2[:], nzt[:], 2.0, -1.0, ALU.mult, ALU.add)
                fcs = fpool.tile([34, FRAMES], F32R, tag="fcs")
                for half in range(2):
                    pm = ps_fft.tile([34, 512], F32, tag="pfft")
                    nc.tensor.matmul(pm[:], fcat_t[:],
                                     nz2[:, 512 * half:512 * (half + 1)],
                                     start=True, stop=True)
                    nc.vector.tensor_tensor(fcs[:, 512 * half:512 * (half + 1)], pm[:],
                                            nl_sb[ex][:, 512 * half:512 * (half + 1)],
                                            ALU.mult)
                frsA = fpool.tile([16, FRAMES], F32, tag="frsA")
                frsB = fpool.tile([16, FRAMES], F32, tag="frsB")
                for half in range(2):
                    pm = ps_fft.tile([34, 512], F32, tag="pfft")
                    nc.tensor.matmul(pm[0:16, :], gmat_t[:, 0:16],
                                     fcs[:, 512 * half:512 * (half + 1)],
                                     start=True, stop=True)
                    nc.scalar.copy(frsA[:, 512 * half:512 * (half + 1)], pm[0:16, :])
                    pm2 = ps_fft.tile([34, 512], F32, tag="pfft")
                    nc.tensor.matmul(pm2[0:16, :], gmat_t[:, 16:32],
                                     fcs[:, 512 * half:512 * (half + 1)],
                                     start=True, stop=True)
                    nc.scalar.copy(frsB[:, 512 * half:512 * (half + 1)], pm2[0:16, :])
                nsb = fpool.tile([16, FRAMES], F32, tag="nsb")
                nc.vector.tensor_copy(nsb[:, 0:1], frsA[:, 0:1])
                nc.vector.tensor_tensor(nsb[:, 1:FRAMES], frsA[:, 1:FRAMES],
                                        frsB[:, 0:FRAMES - 1], ALU.add)
                nc.sync.dma_start(n_out[16 * ex:16 * (ex + 1), :], nsb[:])
                yield

        # round-robin: 3 osc groups per noise chunk, then drain the rest
        osc_list = [(ex, g) for ex in range(BPC) for g in range(len(groups))]
        osc_i = 0
        for _ in noise_gen():
            for _ in range(3):
                if osc_i < len(osc_list):
                    emit_osc_group(*osc_list[osc_i])
                    osc_i += 1
        while osc_i < len(osc_list):
            emit_osc_group(*osc_list[osc_i])
            osc_i += 1

    nc.compile()
    _BUILD_CACHE["nc"] = nc
    return nc


_W_PAT = None


def _wpat():
    global _W_PAT
    if _W_PAT is None:
        w = np.zeros(UP_LEN, np.float32)
        kk = ((np.arange(SEG) + 0.5) / SEG).astype(np.float32)
        for s in range(NSEG):
            w[EDGE + SEG * s: EDGE + SEG * (s + 1)] = kk
        _W_PAT = w
    return _W_PAT


def _prep_shared(inputs):
    d = {}

    def hilo(key, arr):
        a = np.ascontiguousarray(arr).astype(np.float32)
        h = a.astype(np.float16)
        l = (a - h.astype(np.float32)).astype(np.float16)
        d[key + "h"] = np.ascontiguousarray(h)
        d[key + "l"] = np.ascontiguousarray(l)

    hilo("wm0", inputs["w_main0"][:, :, 0].T)
    for i in (1, 2, 3):
        hilo(f"wm{i}", np.asarray(inputs[f"w_main{i}"]).transpose(1, 2, 0))
    hilo("wfq", inputs["w_freq"][:, :, 0].T)
    for l in range(4):
        W = np.asarray(inputs[f"w_nl{l}"])
        We = np.stack([W[:, :, 0], W[:, :, 1] + W[:, :, 2],
                       W[:, :, 3] + W[:, :, 4], W[:, :, 5] + W[:, :, 6]], -1)
        Wo = np.stack([W[:, :, 0] + W[:, :, 1], W[:, :, 2] + W[:, :, 3],
                       W[:, :, 4] + W[:, :, 5], W[:, :, 6]], -1)
        arr = np.stack([We.transpose(1, 2, 0), Wo.transpose(1, 2, 0)], 0)
        # arr: [2eo, cin512, tap4, cout512] -> [2, p128, tap, q2, s2, cout]
        arr = arr.reshape(2, 2, 2, 128, 4, 512)          # [eo, q, s, p, tap, co]
        arr = arr.transpose(0, 3, 4, 1, 2, 5)            # [eo, p, tap, q, s, co]
        a8 = np.ascontiguousarray(arr).astype(ml_dtypes.float8_e4m3)
        d[f"wn8_{l}"] = np.ascontiguousarray(
            a8.reshape(2, 128, 4 * 2 * 2 * 512).view(np.uint8))
    wh = np.asarray(inputs["w_noise_loud"])[:, :, 0].T          # [512, 17]
    d["wnh"] = np.ascontiguousarray(
        np.concatenate([wh, wh], 1).astype(np.float16))
    bn = np.zeros((128, 16), np.float32)
    for l in range(4):
        bl = np.asarray(inputs[f"b_nl{l}"]).reshape(4, 128)
        for m in range(4):
            bn[:, 4 * l + m] = bl[m]
    d["bnl"] = bn
    d["wt"] = np.ascontiguousarray(
        np.broadcast_to(((np.arange(SEG) + 0.5) / SEG).astype(np.float32), (128, SEG)))
    k = np.arange(WIN)[:, None].astype(np.float64)
    j = np.arange(17)[None, :].astype(np.float64)
    fre = np.cos(-2 * np.pi * k * j / WIN) / np.sqrt(WIN)
    fim = np.sin(-2 * np.pi * k * j / WIN) / np.sqrt(WIN)
    d["fcat"] = np.ascontiguousarray(np.concatenate([fre, fim], 1).astype(np.float32))
    t = np.arange(WIN)[None, :].astype(np.float64)
    jj = np.arange(17)[:, None].astype(np.float64)
    wgt = np.where((jj == 0) | (jj == 16), 1.0, 2.0)
    gre = wgt * np.cos(2 * np.pi * jj * t / WIN) / np.sqrt(WIN)
    gim = -wgt * np.sin(2 * np.pi * jj * t / WIN) / np.sqrt(WIN)
    d["gmat"] = np.ascontiguousarray(np.concatenate([gre, gim], 0).astype(np.float32))
    d["cesc"] = (0.5 * _ERBS / SR).astype(np.float32).reshape(128, 1)
    d["cebi"] = (_CENTERS / SR).astype(np.float32).reshape(128, 1)
    return d


def _in_maps(inputs):
    shared = _prep_shared(inputs)
    x = np.asarray(inputs["x"], np.float32)
    noise = np.asarray(inputs["noise"], np.float32)
    maps = []
    for c in range(NCORES):
        m = dict(shared)
        xc = x[BPC * c:BPC * (c + 1)]
        xh = xc.astype(np.float16)
        m["x3h"] = np.ascontiguousarray(xh)
        m["x3l"] = np.ascontiguousarray((xc - xh.astype(np.float32)).astype(np.float16))
        m["noi"] = np.ascontiguousarray(noise[BPC * c:BPC * (c + 1)])
        maps.append(m)
    return maps


def _assemble(results):
    wpat = _wpat()
    groups = _osc_groups()
    out = np.empty((B, 1, TOTAL), np.float32)
    for c in range(NCORES):
        h_o = results[c]["h_out"]       # [BPC, 128, 17*SEG] fp16
        n_o = results[c]["n_out"]
        for ex in range(BPC):
            bidx = BPC * c + ex
            hb = h_o[ex].reshape(128, 17, SEG).astype(np.float32)
            lo = np.empty(UP_LEN, np.float32)
            dl = np.empty(UP_LEN, np.float32)
            for g, (glist, is_edge) in enumerate(groups):
                for gi, (j, c0, wdt) in enumerate(glist):
                    lo[c0:c0 + wdt] = hb[32 * gi, g, 0:wdt]
                    dl[c0:c0 + wdt] = hb[32 * gi + 1, g, 0:wdt]
            sig = lo + wpat * dl
            nzf = np.ascontiguousarray(n_o[16 * ex:16 * (ex + 1)].T).reshape(TOTAL)
            sig[CROP:CROP + TOTAL] += nzf
            mx = np.abs(sig).max()
            out[bidx, 0] = sig[CROP:CROP + TOTAL] / (mx + np.float32(1e-8))
    return out


def kernel(**inputs) -> np.ndarray:
    nc = _build_program()
    maps = _in_maps(inputs)
    res = bass_utils.run_bass_kernel_spmd(nc, maps, core_ids=list(range(NCORES)))
    return _assemble([res.results[c] for c in range(NCORES)])


# revision 20
# speedup vs baseline: 1.0481x; 1.0444x over previous
"""DDSP generator Bass kernel for Trainium2, 8-core data parallel.

Sharding: batch 16 -> 8 cores x 2 examples each. Weights replicated.
Per core:
  stage1: main conv stack (fp32 PE) -> h; osc head -> l (amp^2), f (Hz/SR)
  osc bank, per 384-sample resize segment (plus two 192 edge segments):
      lerp (DVE/Pool tensor_scalar, per-partition scale/bias) ->
      custom DVE op (clip + cumsum + wrap to [-0.5, 0.5] cycles, one pass) ->
      ACT Sin (per group of 4 units) -> fp16 -> m=2 PE reduce matmul with
      lhsT = [l_lo | dl], 4 segments packed per PSUM bank via tile_position ->
      direct PSUM -> DRAM DMA (partition-strided, one per group).
  noise branch: 4x (2x-upsample conv k7) via even/odd stride trick
      (host-combined 4-tap weights), fp8e4m3 DoubleRow matmuls (0.5 cyc/row,
      contraction 256); activations quantized to fp8 between layers; last
      layer kept fp16 for the head conv (duplicated 34-col weights) +
      Square -> n_l on partitions 0..33.
  noise FFT: rfft/irfft as fp32r DFT matmuls, filter on DVE, overlap-add.
Host: recombine the two reduce rows with the lerp-weight pattern, pad,
      add noise, normalize, crop (O(output) numpy work only).
"""

import numpy as np
from contextlib import ExitStack

import ml_dtypes

import concourse.bass as bass
import concourse.tile as tile
from concourse import bacc, mybir
from concourse import bass_utils
from concourse import dve_ops
from concourse.dve_spec import Spec, Src0, Src1, C0, C1, C2, scan, minn, maxx, AluOp, lower
from concourse.dve_uop import DveOpSpec

F32 = mybir.dt.float32
F16 = mybir.dt.float16
F8 = mybir.dt.float8e4
U8 = mybir.dt.uint8
F32R = mybir.dt.float32r
AF = mybir.ActivationFunctionType
ALU = mybir.AluOpType
DR = mybir.MatmulPerfMode.DoubleRow

SR = 11025.0
UP_LEN = 24576
TOTAL = 16384
WIN = 32
FRAMES = 1024
CROP = 4096
B = 16
NCORES = 8
BPC = 2
T0 = 64
SEG = 384
NSEG = 63
EDGE = 192
NUNITS = NSEG + 2
LO_U = 20.0 / SR
HI_U = 0.5
MAGIC = 12582912.0

_CENTERS = np.geomspace(20.0, SR / 2.0 - 20.0, 128).astype(np.float32)
_ERBS = (_CENTERS * np.float32(0.108) + np.float32(24.7)).astype(np.float32)


def _osc_ref(in0, in1, s0, s1, imm2):
    v = np.minimum(np.maximum(in0, np.float32(s0)), np.float32(s1)).astype(np.float32)
    u = np.cumsum(v.astype(np.float64), axis=-1).astype(np.float32)
    y = (u + in1).astype(np.float32)
    r = ((y + np.float32(imm2)) - np.float32(imm2)).astype(np.float32)
    return (y - r).astype(np.float32)


def _register_osc_op():
    if hasattr(dve_ops, "CUSTOM_DVE_OPS_BY_NAME") and \
            "OSC_PHASE_ANT" in dve_ops.CUSTOM_DVE_OPS_BY_NAME:
        return dve_ops.CUSTOM_DVE_OPS_BY_NAME["OSC_PHASE_ANT"]
    body_v = minn(maxx(Src0, C0), C1)
    body_u = scan(AluOp.ADD, body_v)
    body_y = body_u + Src1
    body = body_y - ((body_y + C2) - C2)
    spec = Spec(body=body, reference=_osc_ref)
    sha = {}
    for ver in ("v3",):
        s = DveOpSpec(name="OSC_PHASE_ANT", opcode=1, uops=lower(spec, ver=ver),
                      rd1_en=True)
        sha[ver] = s.sha(ver)
    op = dve_ops.DveOp("OSC_PHASE_ANT", spec, subdim=False, uops_sha=sha)
    dve_ops.OPS.append(op)
    dve_ops.CUSTOM_DVE_SPECS[op.name] = op.spec
    dve_ops._SUB_OPCODE_FOR_NAME[op.name] = max(dve_ops._SUB_OPCODE_FOR_NAME.values()) + 1
    if not hasattr(dve_ops, "CUSTOM_DVE_OPS_BY_NAME"):
        dve_ops.CUSTOM_DVE_OPS_BY_NAME = {}
    dve_ops.CUSTOM_DVE_OPS_BY_NAME[op.name] = op
    return op


def _osc_groups():
    """Per-example unit grouping: 16 groups of SEG units (last has 3) plus
    one edge group [unit0, unit64]. Returns list of (glist, is_edge) where
    glist entries are (j, c0, wdt)."""
    groups = []
    for g in range(16):
        js = [1 + 4 * g + i for i in range(4) if 1 + 4 * g + i <= 63]
        groups.append(([(j, EDGE + SEG * (j - 1), SEG) for j in js], False))
    groups.append(([(0, 0, EDGE), (NUNITS - 1, UP_LEN - EDGE, EDGE)], True))
    return groups


_BUILD_CACHE = {}


def _build_program():
    if "nc" in _BUILD_CACHE:
        return _BUILD_CACHE["nc"]
    osc_op = _register_osc_op()

    nc = bacc.Bacc("TRN2", target_bir_lowering=False, debug=False, num_devices=1)

    dI = lambda n, s, dt=F32: nc.dram_tensor(n, s, dt, kind="ExternalInput").ap()
    dO = lambda n, s, dt=F32: nc.dram_tensor(n, s, dt, kind="ExternalOutput").ap()

    x3h = dI("x3h", [BPC, 256, T0], F16)
    x3l = dI("x3l", [BPC, 256, T0], F16)
    noi = dI("noi", [BPC, FRAMES, WIN])
    wm0 = [dI(f"wm0{s}", [256, 512], F16) for s in "hl"]
    wmL = [[dI(f"wm{i}{s}", [512, 3, 512], F16) for s in "hl"] for i in (1, 2, 3)]
    wfq = [dI(f"wfq{s}", [512, 256], F16) for s in "hl"]
    wn8 = [dI(f"wn8_{l}", [2, 128, 4 * 2 * 2 * 512], U8) for l in range(4)]
    wnh = dI("wnh", [512, 34], F16)                       # head, duplicated cols
    bnl = dI("bnl", [128, 16])
    wt = dI("wt", [128, SEG])
    fcat = dI("fcat", [WIN, 34])
    gmat = dI("gmat", [34, WIN])
    cesc = dI("cesc", [128, 1])
    cebi = dI("cebi", [128, 1])

    h_out = dO("h_out", [BPC, 128, 17 * SEG], F16)
    n_out = dO("n_out", [16 * BPC, FRAMES])

    with tile.TileContext(nc) as tc, ExitStack() as ctx:
        cpool = ctx.enter_context(tc.tile_pool(name="consts", bufs=1))
        apool = ctx.enter_context(tc.tile_pool(name="acts", bufs=1))
        fpool = ctx.enter_context(tc.tile_pool(name="fft", bufs=1))
        opool = ctx.enter_context(tc.tile_pool(name="osc", bufs=1))
        w1pool = ctx.enter_context(tc.tile_pool(name="w1", bufs=2))
        w2pool = ctx.enter_context(tc.tile_pool(name="w2", bufs=3))
        ps_mm = ctx.enter_context(tc.tile_pool(name="psmm", bufs=4, space="PSUM"))
        ps_osc = ctx.enter_context(tc.tile_pool(name="psosc", bufs=2, space="PSUM"))
        ps_fft = ctx.enter_context(tc.tile_pool(name="psfft", bufs=2, space="PSUM"))

        # ================= stage 1: main conv stack =================
        x_t = []
        for k in range(2):
            xth = apool.tile([128, BPC, T0], F16, tag=f"xh{k}")
            nc.sync.dma_start(xth[:], x3h[:, 128 * k:128 * (k + 1), :].rearrange("b c t -> c b t"))
            xtl = apool.tile([128, BPC, T0], F16, tag=f"xl{k}")
            nc.sync.dma_start(xtl[:], x3l[:, 128 * k:128 * (k + 1), :].rearrange("b c t -> c b t"))
            x_t.append((xth, xtl))

        wm0_t = []
        for k in range(2):
            wh = w1pool.tile([128, 512], F16, tag=f"wm0h_{k}")
            nc.sync.dma_start(wh[:], wm0[0][128 * k:128 * (k + 1), :])
            wl_ = w1pool.tile([128, 512], F16, tag=f"wm0l_{k}")
            nc.sync.dma_start(wl_[:], wm0[1][128 * k:128 * (k + 1), :])
            wm0_t.append((wh, wl_))

        wt_t = cpool.tile([128, SEG], F32)
        nc.sync.dma_start(wt_t[:], wt[:])
        cesc_t = cpool.tile([128, 1], F32)
        nc.sync.dma_start(cesc_t[:], cesc[:])
        cebi_t = cpool.tile([128, 1], F32)
        nc.sync.dma_start(cebi_t[:], cebi[:])
        fcat_t = cpool.tile([WIN, 34], F32R)
        nc.sync.dma_start(fcat_t[:], fcat[:].bitcast(F32R))
        gmat_t = cpool.tile([34, WIN], F32R)
        nc.sync.dma_start(gmat_t[:], gmat[:].bitcast(F32R))
        bnl_t = cpool.tile([128, 16], F32)
        nc.sync.dma_start(bnl_t[:], bnl[:])

        NCOL = BPC * T0
        h1 = []
        for m in range(4):
            pm = ps_mm.tile([128, 512], F32, tag="pconv")
            i_mm = 0
            for k in range(2):
                ms = slice(128 * m, 128 * (m + 1))
                for lh, rh in ((0, 0), (0, 1), (1, 0)):
                    nc.tensor.matmul(pm[:, 0:NCOL], wm0_t[k][lh][:, ms],
                                     x_t[k][rh][:],
                                     start=(i_mm == 0), stop=(i_mm == 5))
                    i_mm += 1
            ht = apool.tile([128, BPC, 66], F32, tag=f"hA{m}")
            nc.gpsimd.memset(ht[:, :, 0:1], 0.0)
            nc.gpsimd.memset(ht[:, :, 65:66], 0.0)
            nc.scalar.activation(ht[:, :, 1:65],
                                 pm[:, 0:NCOL].rearrange("c (b t) -> c b t", b=BPC),
                                 AF.Prelu, bias=0.0, scale=1.0, alpha=0.2)
            h1.append(ht)

        def split16(tiles, PAD, WID, tagp):
            # fp32 h tiles -> (hi, lo) fp16 pairs, pads included (zero)
            out = []
            for k, ht in enumerate(tiles):
                hh = apool.tile([128, BPC, WID], F16, tag=f"{tagp}h{k}")
                nc.vector.tensor_copy(hh[:], ht[:])
                hl = apool.tile([128, BPC, WID], F16, tag=f"{tagp}l{k}")
                nc.vector.tensor_tensor(hl[:], ht[:], hh[:], ALU.subtract)
                out.append((hh, hl))
            return out

        hcur = split16(h1, 1, 66, "sA")
        for li in range(3):
            wl = []
            for k in range(4):
                wh = w1pool.tile([128, 3 * 512], F16, tag=f"wmLh_{k}")
                nc.sync.dma_start(wh[:], wmL[li][0][128 * k:128 * (k + 1), :, :]
                                  .rearrange("c a o -> c (a o)"))
                wlo = w1pool.tile([128, 3 * 512], F16, tag=f"wmLl_{k}")
                nc.sync.dma_start(wlo[:], wmL[li][1][128 * k:128 * (k + 1), :, :]
                                  .rearrange("c a o -> c (a o)"))
                wl.append((wh, wlo))
            last = li == 2
            PAD = 2 if last else 1
            WID = T0 + 2 * PAD
            tagp = "hB" if li % 2 == 0 else "hA"
            hnxt = []
            for m in range(4):
                pm = ps_mm.tile([128, 512], F32, tag="pconv")
                i_mm = 0
                for k in range(4):
                    for tap in range(3):
                        wsl = slice(512 * tap + 128 * m, 512 * tap + 128 * (m + 1))
                        for lh, rh in ((0, 0), (0, 1), (1, 0)):
                            nc.tensor.matmul(
                                pm[:, 0:NCOL],
                                wl[k][lh][:, wsl],
                                hcur[k][rh][:, :, tap:tap + T0],
                                start=(i_mm == 0), stop=(i_mm == 35))
                            i_mm += 1
                ht = apool.tile([128, BPC, WID], F32,
                                tag=(f"h4_{m}" if last else f"{tagp}{m}"))
                nc.gpsimd.memset(ht[:, :, 0:PAD], 0.0)
                nc.gpsimd.memset(ht[:, :, PAD + T0:WID], 0.0)
                nc.scalar.activation(ht[:, :, PAD:PAD + T0],
                                     pm[:, 0:NCOL].rearrange("c (b t) -> c b t", b=BPC),
                                     AF.Prelu, bias=0.0, scale=1.0, alpha=0.2)
                hnxt.append(ht)
            hcur = split16(hnxt, PAD, WID, "sB" if li % 2 == 0 else "sA")
            if last:
                h4 = hnxt   # 4 x [128, BPC, 68] fp32, pad 2
        h4s = hcur

        wfq_t = []
        for k in range(4):
            wh = w1pool.tile([128, 256], F16, tag=f"wfqh{k}")
            nc.sync.dma_start(wh[:], wfq[0][128 * k:128 * (k + 1), :])
            wlo = w1pool.tile([128, 256], F16, tag=f"wfql{k}")
            nc.sync.dma_start(wlo[:], wfq[1][128 * k:128 * (k + 1), :])
            wfq_t.append((wh, wlo))
        l_sb = apool.tile([128, BPC, T0], F32, tag="l_sb")
        f_sb = apool.tile([128, BPC, T0], F32, tag="f_sb")
        for m in range(2):
            pm = ps_mm.tile([128, 512], F32, tag="pconv")
            i_mm = 0
            for k in range(4):
                ms = slice(128 * m, 128 * (m + 1))
                for lh, rh in ((0, 0), (0, 1), (1, 0)):
                    nc.tensor.matmul(pm[:, 0:NCOL], wfq_t[k][lh][:, ms],
                                     h4s[k][rh][:, :, 2:2 + T0],
                                     start=(i_mm == 0), stop=(i_mm == 11))
                    i_mm += 1
            if m == 0:
                nc.scalar.activation(l_sb[:],
                                     pm[:, 0:NCOL].rearrange("c (b t) -> c b t", b=BPC),
                                     AF.Square)
            else:
                tanh_t = apool.tile([128, BPC, T0], F32, tag="tanh")
                nc.scalar.activation(tanh_t[:],
                                     pm[:, 0:NCOL].rearrange("c (b t) -> c b t", b=BPC),
                                     AF.Tanh)
                nc.scalar.activation(f_sb[:], tanh_t[:],
                                     AF.Identity, bias=cebi_t[:], scale=cesc_t[:])

        # ================= osc prep =================
        flo_u, df_u, c_u, l2_u = [], [], [], []
        for ex in range(BPC):
            f_ex = f_sb[:, ex, :]
            l_ex = l_sb[:, ex, :]

            flo = apool.tile([128, NUNITS], F32, tag=f"flo{ex}")
            nc.vector.tensor_copy(flo[:, 0:1], f_ex[:, 0:1])
            nc.vector.tensor_copy(flo[:, 1:65], f_ex[:, 0:64])
            dfu = apool.tile([128, NUNITS], F32, tag=f"dfu{ex}")
            nc.gpsimd.memset(dfu[:, 0:1], 0.0)
            nc.gpsimd.memset(dfu[:, 64:65], 0.0)
            nc.gpsimd.tensor_tensor(dfu[:, 1:64], f_ex[:, 1:64], f_ex[:, 0:63], ALU.subtract)

            l2t = apool.tile([128, NUNITS, 2], F16, tag=f"l2{ex}")
            nc.vector.tensor_copy(l2t[:, 0:1, 0], l_ex[:, 0:1])
            nc.vector.tensor_copy(l2t[:, 1:65, 0], l_ex[:, 0:64])
            nc.gpsimd.memset(l2t[:, 0:1, 1], 0.0)
            nc.gpsimd.memset(l2t[:, 64:65, 1], 0.0)
            nc.gpsimd.tensor_tensor(l2t[:, 1:64, 1], l_ex[:, 1:64], l_ex[:, 0:63], ALU.subtract)

            a = f_ex[:, 0:63]
            b_ = f_ex[:, 1:64]

            def T63(tag):
                return apool.tile([128, 63], F32, tag=tag, name=tag)

            alo = T63("p_alo")
            nc.vector.tensor_tensor(alo[:], a, b_, ALU.min)
            ahi = T63("p_ahi")
            nc.vector.tensor_tensor(ahi[:], a, b_, ALU.max)
            dd = T63("p_dd")
            nc.vector.tensor_tensor(dd[:], ahi[:], alo[:], ALU.subtract)
            ddc = T63("p_ddc")
            nc.vector.tensor_scalar(ddc[:], dd[:], 1e-30, None, ALU.max)
            inv = T63("p_inv")
            nc.vector.reciprocal(inv[:], ddc[:])
            dd768 = T63("p_dd768")
            nc.vector.tensor_scalar(dd768[:], dd[:], float(1.0 / 768.0), None, ALU.mult)

            t1 = T63("p_t1")
            nc.vector.tensor_scalar(t1[:], alo[:], LO_U, -384.0, ALU.subtract, ALU.mult)
            c1 = T63("p_c1")
            nc.vector.tensor_tensor(c1[:], t1[:], inv[:], ALU.mult)
            nc.vector.tensor_scalar(c1[:], c1[:], 0.0, 384.0, ALU.max, ALU.min)
            nc.vector.tensor_scalar(c1[:], c1[:], MAGIC, MAGIC, ALU.add, ALU.subtract)
            lo_alo = T63("p_loalo")
            nc.vector.tensor_scalar(lo_alo[:], alo[:], LO_U, -1.0, ALU.subtract, ALU.mult)
            u1 = T63("p_u1")
            nc.vector.tensor_tensor(u1[:], dd768[:], c1[:], ALU.mult)
            nc.vector.tensor_tensor(u1[:], lo_alo[:], u1[:], ALU.subtract)
            s1c = T63("p_s1c")
            nc.vector.tensor_tensor(s1c[:], c1[:], u1[:], ALU.mult)

            t2 = T63("p_t2")
            nc.vector.tensor_scalar(t2[:], ahi[:], HI_U, 384.0, ALU.subtract, ALU.mult)
            c2 = T63("p_c2")
            nc.vector.tensor_tensor(c2[:], t2[:], inv[:], ALU.mult)
            nc.vector.tensor_scalar(c2[:], c2[:], 0.0, 384.0, ALU.max, ALU.min)
            nc.vector.tensor_scalar(c2[:], c2[:], MAGIC, MAGIC, ALU.add, ALU.subtract)
            ahi_hi = T63("p_ahihi")
            nc.vector.tensor_scalar(ahi_hi[:], ahi[:], HI_U, None, ALU.subtract)
            u2 = T63("p_u2")
            nc.vector.tensor_tensor(u2[:], dd768[:], c2[:], ALU.mult)
            nc.vector.tensor_tensor(u2[:], ahi_hi[:], u2[:], ALU.subtract)
            s2c = T63("p_s2c")
            nc.vector.tensor_tensor(s2c[:], c2[:], u2[:], ALU.mult)

            tall = apool.tile([128, 64], F32, tag="p_tall")
            slin = T63("p_slin")
            nc.vector.tensor_tensor(slin[:], a, b_, ALU.add)
            nc.vector.tensor_scalar(slin[:], slin[:], 192.0, None, ALU.mult)
            nc.vector.tensor_tensor(tall[:, 1:64], slin[:], s1c[:], ALU.add)
            nc.vector.tensor_tensor(tall[:, 1:64], tall[:, 1:64], s2c[:], ALU.subtract)
            nc.vector.tensor_scalar(tall[:, 0:1], f_ex[:, 0:1], LO_U, HI_U, ALU.max, ALU.min)
            nc.vector.tensor_scalar(tall[:, 0:1], tall[:, 0:1], 192.0, None, ALU.mult)
            trnd = apool.tile([128, 64], F32, tag="p_trnd")
            nc.vector.tensor_scalar(trnd[:], tall[:], MAGIC, MAGIC, ALU.add, ALU.subtract)
            nc.vector.tensor_tensor(tall[:], tall[:], trnd[:], ALU.subtract)
            cinc = apool.tile([128, 64], F32, tag="p_cinc")
            nc.vector.tensor_tensor_scan(cinc[:], tall[:], tall[:], 0.0, ALU.add, ALU.bypass)
            cu = apool.tile([128, NUNITS], F32, tag=f"cu{ex}")
            nc.gpsimd.memset(cu[:, 0:1], 0.0)
            nc.vector.tensor_copy(cu[:, 1:65], cinc[:])

            flo_u.append(flo)
            df_u.append(dfu)
            c_u.append(cu)
            l2_u.append(l2t)

        # ================= osc bank + noise branch, interleaved =================
        # Engines execute their queues in order, so emission order controls
        # overlap: alternate DVE-heavy osc groups with PE-heavy noise chunks.
        two_pi = float(2.0 * np.pi)
        groups = _osc_groups()
        lerp_state = [0]
        hm_state = [0]
        osc_done = [0, 0]
        hm_chunk = [None]

        def emit_osc_group(ex, g):
            glist, _ = groups[g]
            pm4 = ps_osc.tile([128, SEG], F32, tag="pm4")
            ph4 = opool.tile([128, 4 * SEG], F32, tag="ph4", bufs=2)
            s16 = opool.tile([128, 4 * SEG], F16, tag="s16", bufs=3)
            offs = []
            cur = 0
            for (j, c0, wdt) in glist:
                offs.append(cur)
                cur += wdt
            used = cur
            for gi, (j, c0, wdt) in enumerate(glist):
                fu = opool.tile([128, SEG], F32, tag="fu", bufs=4)
                eng = nc.vector if lerp_state[0] % 5 < 2 else nc.gpsimd
                eng.tensor_scalar(fu[:, 0:wdt], wt_t[:, 0:wdt],
                                  df_u[ex][:, j:j + 1],
                                  flo_u[ex][:, j:j + 1],
                                  ALU.mult, ALU.add)
                lerp_state[0] += 1
                nc.vector._custom_dve(
                    osc_op, out=ph4[:, offs[gi]:offs[gi] + wdt], in0=fu[:, 0:wdt],
                    in1=c_u[ex][:, j:j + 1].to_broadcast((128, wdt)),
                    s0=LO_U, s1=HI_U, imm2=MAGIC)
            nc.scalar.activation(s16[:, 0:used], ph4[:, 0:used], AF.Sin,
                                 bias=0.0, scale=two_pi)
            for gi, (j, c0, wdt) in enumerate(glist):
                nc.tensor.matmul(pm4[32 * gi:32 * gi + 2, 0:wdt],
                                 l2_u[ex][:, j, :],
                                 s16[:, offs[gi]:offs[gi] + wdt],
                                 start=True, stop=True,
                                 tile_position=(0, 32 * gi))
            wdma = glist[0][2]
            if g % 4 == 0:
                hm_chunk[0] = opool.tile([128, 4, SEG], F16, tag="hmb", bufs=3, name="hmb")
            hmb = hm_chunk[0]
            slot = g % 4
            if hm_state[0] % 6 == 5:
                nc.vector.tensor_copy(hmb[:, slot, 0:wdma], pm4[:, 0:wdma])
            else:
                nc.scalar.copy(hmb[:, slot, 0:wdma], pm4[:, 0:wdma])
            hm_state[0] += 1
            osc_done[ex] += 1
            if slot == 3 or g == len(groups) - 1:
                g0 = g - slot
                nc.sync.dma_start(
                    h_out[ex, :, g0 * SEG:(g + 1) * SEG],
                    hmb[:, 0:slot + 1, :].rearrange("c g s -> c (g s)"))

        def noise_gen():
            # input pairs from h4 (fp32 -> fp8)
            hp = []
            for q in range(2):
                t8 = apool.tile([128, 2, BPC, 68], F8, tag=f"hp{q}", name=f"hp{q}")
                for s in range(2):
                    nc.vector.tensor_copy(t8[:, s], h4[2 * q + s][:])
                hp.append(t8)

            ycur = hp          # pair tiles, fp8
            TI = T0
            for li in range(4):
                TOUT = TI * 2
                WIDO = TOUT + 4
                last = li == 3
                odt = F16 if last else F8
                tagp = "yA" if li % 2 == 0 else "yB"
                ynxt = [apool.tile([128, 2, BPC, WIDO], odt, tag=f"{tagp}{q}",
                                   name=f"{tagp}{q}") for q in range(2)]
                for q in range(2):
                    nc.gpsimd.memset(ynxt[q][:, :, :, 0:2], 0.0)
                    nc.gpsimd.memset(ynxt[q][:, :, :, WIDO - 2:WIDO], 0.0)
                per_ex = BPC * TI > 512
                for eo in range(2):
                    wgt = w2pool.tile([128, 4, 2, 2, 512], F8, tag="wn8")
                    nc.sync.dma_start(wgt[:].bitcast(U8)
                                      .rearrange("c a q s o -> c (a q s o)"),
                                      wn8[li][eo, :, :])
                    for m in range(4):
                        bias_ap = bnl_t[:, 4 * li + m:4 * li + m + 1]
                        ex_sets = list(range(BPC)) if per_ex else [None]
                        for ex in ex_sets:
                            ncols = TI if per_ex else BPC * TI
                            pm = ps_mm.tile([128, 512], F32, tag="pconv")
                            i_mm = 0
                            for q in range(2):
                                for tap in range(4):
                                    off = tap + eo
                                    if per_ex:
                                        rhs = ycur[q][:, :, ex, off:off + TI]
                                    else:
                                        rhs = ycur[q][:, :, :, off:off + TI]
                                    lhsT = wgt[:, tap, q, :, 128 * m:128 * (m + 1)]
                                    nc.tensor.matmul(pm[:, 0:ncols], lhsT, rhs,
                                                     start=(i_mm == 0), stop=(i_mm == 7),
                                                     perf_mode=DR)
                                    i_mm += 1
                            if per_ex:
                                dst = ynxt[m // 2][:, m % 2, ex, 2 + eo:2 + eo + 2 * TI:2]
                                srcp = pm[:, 0:ncols]
                            else:
                                dst = ynxt[m // 2][:, m % 2, :, 2 + eo:2 + eo + 2 * TI:2]
                                srcp = pm[:, 0:ncols].rearrange("c (b t) -> c b t", b=BPC)
                            nc.scalar.activation(dst, srcp, AF.Prelu,
                                                 bias=bias_ap, scale=1.0, alpha=0.2)
                    yield
                ycur = ynxt
                TI = TOUT

            wh_t = w2pool.tile([128, 4, 34], F16, tag="wnh")
            for k in range(4):
                nc.sync.dma_start(wh_t[:, k, :], wnh[128 * k:128 * (k + 1), :])
            nl_sb = []
            for ex in range(BPC):
                nlt = apool.tile([34, FRAMES], F32, tag=f"nl{ex}")
                for half in range(2):
                    pm = ps_fft.tile([34, 512], F32, tag="pfft")
                    for k in range(4):
                        nc.tensor.matmul(pm[:],
                                         wh_t[:, k, :],
                                         ycur[k // 2][:, k % 2, ex,
                                                      2 + 512 * half:2 + 512 * (half + 1)],
                                         start=(k == 0), stop=(k == 3))
                    nc.scalar.activation(nlt[:, 512 * half:512 * (half + 1)], pm[:], AF.Square)
                nl_sb.append(nlt)
            yield

            # ---- noise FFT ----
            for ex in range(BPC):
                nzt = fpool.tile([WIN, FRAMES], F32, tag="nz")
                nc.sync.dma_start_transpose(nzt[:], noi[ex, :, :])
                nz2 = fpool.tile([WIN, FRAMES], F32R, tag="nz2")
                nc.vector.tensor_scalar(nz2[:], nzt[:], 2.0, -1.0, ALU.mult, ALU.add)
                fcs = fpool.tile([34, FRAMES], F32R, tag="fcs")
                for half in range(2):
                    pm = ps_fft.tile([34, 512], F32, tag="pfft")
                    nc.tensor.matmul(pm[:], fcat_t[:],
                                     nz2[:, 512 * half:512 * (half + 1)],
                                     start=True, stop=True)
                    nc.vector.tensor_tensor(fcs[:, 512 * half:512 * (half + 1)], pm[:],
                                            nl_sb[ex][:, 512 * half:512 * (half + 1)],
                                            ALU.mult)
                frsA = fpool.tile([16, FRAMES], F32, tag="frsA")
                frsB = fpool.tile([16, FRAMES], F32, tag="frsB")
                for half in range(2):
                    pm = ps_fft.tile([34, 512], F32, tag="pfft")
                    nc.tensor.matmul(pm[0:16, :], gmat_t[:, 0:16],
                                     fcs[:, 512 * half:512 * (half + 1)],
                                     start=True, stop=True)
                    nc.scalar.copy(frsA[:, 512 * half:512 * (half + 1)], pm[0:16, :])
                    pm2 = ps_fft.tile([34, 512], F32, tag="pfft")
                    nc.tensor.matmul(pm2[0:16, :], gmat_t[:, 16:32],
                                     fcs[:, 512 * half:512 * (half + 1)],
                                     start=True, stop=True)
                    nc.scalar.copy(frsB[:, 512 * half:512 * (half + 1)], pm2[0:16, :])
                nsb = fpool.tile([16, FRAMES], F32, tag="nsb")
                nc.vector.tensor_copy(nsb[:, 0:1], frsA[:, 0:1])
                nc.vector.tensor_tensor(nsb[:, 1:FRAMES], frsA[:, 1:FRAMES],
                                        frsB[:, 0:FRAMES - 1], ALU.add)
                nc.sync.dma_start(n_out[16 * ex:16 * (ex + 1), :], nsb[:])
                yield

        # round-robin: 3 osc groups per noise chunk, then drain the rest
        osc_list = [(ex, g) for ex in range(BPC) for g in range(len(groups))]
        osc_i = 0
        for _ in noise_gen():
            for _ in range(3):
                if osc_i < len(osc_list):
                    emit_osc_group(*osc_list[osc_i])
                    osc_i += 1
        while osc_i < len(osc_list):
            emit_osc_group(*osc_list[osc_i])
            osc_i += 1

    nc.compile()
    _BUILD_CACHE["nc"] = nc
    return nc


_W_PAT = None


def _wpat():
    global _W_PAT
    if _W_PAT is None:
        w = np.zeros(UP_LEN, np.float32)
        kk = ((np.arange(SEG) + 0.5) / SEG).astype(np.float32)
        for s in range(NSEG):
            w[EDGE + SEG * s: EDGE + SEG * (s + 1)] = kk
        _W_PAT = w
    return _W_PAT


def _prep_shared(inputs):
    d = {}

    def hilo(key, arr):
        a = np.ascontiguousarray(arr).astype(np.float32)
        h = a.astype(np.float16)
        l = (a - h.astype(np.float32)).astype(np.float16)
        d[key + "h"] = np.ascontiguousarray(h)
        d[key + "l"] = np.ascontiguousarray(l)

    hilo("wm0", inputs["w_main0"][:, :, 0].T)
    for i in (1, 2, 3):
        hilo(f"wm{i}", np.asarray(inputs[f"w_main{i}"]).transpose(1, 2, 0))
    hilo("wfq", inputs["w_freq"][:, :, 0].T)
    for l in range(4):
        W = np.asarray(inputs[f"w_nl{l}"])
        We = np.stack([W[:, :, 0], W[:, :, 1] + W[:, :, 2],
                       W[:, :, 3] + W[:, :, 4], W[:, :, 5] + W[:, :, 6]], -1)
        Wo = np.stack([W[:, :, 0] + W[:, :, 1], W[:, :, 2] + W[:, :, 3],
                       W[:, :, 4] + W[:, :, 5], W[:, :, 6]], -1)
        arr = np.stack([We.transpose(1, 2, 0), Wo.transpose(1, 2, 0)], 0)
        # arr: [2eo, cin512, tap4, cout512] -> [2, p128, tap, q2, s2, cout]
        arr = arr.reshape(2, 2, 2, 128, 4, 512)          # [eo, q, s, p, tap, co]
        arr = arr.transpose(0, 3, 4, 1, 2, 5)            # [eo, p, tap, q, s, co]
        a8 = np.ascontiguousarray(arr).astype(ml_dtypes.float8_e4m3)
        d[f"wn8_{l}"] = np.ascontiguousarray(
            a8.reshape(2, 128, 4 * 2 * 2 * 512).view(np.uint8))
    wh = np.asarray(inputs["w_noise_loud"])[:, :, 0].T          # [512, 17]
    d["wnh"] = np.ascontiguousarray(
        np.concatenate([wh, wh], 1).astype(np.float16))
    bn = np.zeros((128, 16), np.float32)
    for l in range(4):
        bl = np.asarray(inputs[f"b_nl{l}"]).reshape(4, 128)
        for m in range(4):
            bn[:, 4 * l + m] = bl[m]
    d["bnl"] = bn
    d["wt"] = np.ascontiguousarray(
        np.broadcast_to(((np.arange(SEG) + 0.5) / SEG).astype(np.float32), (128, SEG)))
    k = np.arange(WIN)[:, None].astype(np.float64)
    j = np.arange(17)[None, :].astype(np.float64)
    fre = np.cos(-2 * np.pi * k * j / WIN) / np.sqrt(WIN)
    fim = np.sin(-2 * np.pi * k * j / WIN) / np.sqrt(WIN)
    d["fcat"] = np.ascontiguousarray(np.concatenate([fre, fim], 1).astype(np.float32))
    t = np.arange(WIN)[None, :].astype(np.float64)
    jj = np.arange(17)[:, None].astype(np.float64)
    wgt = np.where((jj == 0) | (jj == 16), 1.0, 2.0)
    gre = wgt * np.cos(2 * np.pi * jj * t / WIN) / np.sqrt(WIN)
    gim = -wgt * np.sin(2 * np.pi * jj * t / WIN) / np.sqrt(WIN)
    d["gmat"] = np.ascontiguousarray(np.concatenate([gre, gim], 0).astype(np.float32))
    d["cesc"] = (0.5 * _ERBS / SR).astype(np.float32).reshape(128, 1)
    d["cebi"] = (_CENTERS / SR).astype(np.float32).reshape(128, 1)
    return d


def _in_maps(inputs):
    shared = _prep_shared(inputs)
    x = np.asarray(inputs["x"], np.float32)
    noise = np.asarray(inputs["noise"], np.float32)
    maps = []
    for c in range(NCORES):
        m = dict(shared)
        xc = x[BPC * c:BPC * (c + 1)]
        xh = xc.astype(np.float16)
        m["x3h"] = np.ascontiguousarray(xh)
        m["x3l"] = np.ascontiguousarray((xc - xh.astype(np.float32)).astype(np.float16))
        m["noi"] = np.ascontiguousarray(noise[BPC * c:BPC * (c + 1)])
        maps.append(m)
    return maps


def _assemble(results):
    wpat = _wpat()
    groups = _osc_groups()
    out = np.empty((B, 1, TOTAL), np.float32)
    for c in range(NCORES):
        h_o = results[c]["h_out"]       # [BPC, 128, 17*SEG] fp16
        n_o = results[c]["n_out"]
        for ex in range(BPC):
            bidx = BPC * c + ex
            hb = h_o[ex].reshape(128, 17, SEG).astype(np.float32)
            lo = np.empty(UP_LEN, np.float32)
            dl = np.empty(UP_LEN, np.float32)
            for g, (glist, is_edge) in enumerate(groups):
                for gi, (j, c0, wdt) in enumerate(glist):
                    lo[c0:c0 + wdt] = hb[32 * gi, g, 0:wdt]
                    dl[c0:c0 + wdt] = hb[32 * gi + 1, g, 0:wdt]
            sig = lo + wpat * dl
            nzf = np.ascontiguousarray(n_o[16 * ex:16 * (ex + 1)].T).reshape(TOTAL)
            sig[CROP:CROP + TOTAL] += nzf
            mx = np.abs(sig).max()
            out[bidx, 0] = sig[CROP:CROP + TOTAL] / (mx + np.float32(1e-8))
    return out


def kernel(**inputs) -> np.ndarray:
    nc = _build_program()
    maps = _in_maps(inputs)
    res = bass_utils.run_bass_kernel_spmd(nc, maps, core_ids=list(range(NCORES)))
    return _assemble([res.results[c] for c in range(NCORES)])
